# revision 2
# baseline (speedup 1.0000x reference)
"""Trainium2 Bass kernel v2 for nn_MultiHeadedAttention_4269197492266.

Same math as v1 (folded 5-tap conv local-key path, batch x head-group
sharding, ones-column softmax denominator), restructured around the
cost model:

- Score chunks stream continuously into alternating [128,2048]/[128,1024]
  PSUM staging tiles; exp runs as 170 wide ACT instructions (~250us, the
  ACT floor for 33.5M exps/core).  All other PE work (projections, conv
  taps, AV, transposes, outproj) is queued as sub-microsecond quanta and
  popped between staging tiles under a cycle budget with emission
  deadlines, so the in-order PE queue never starves ACT.
- AV is reoriented: out[lq, dk+1] = e_chunk^T @ V -- each matmul streams
  65 columns instead of 512, halving AV cost.  Group-major score order
  makes AV accumulators sequential: one PSUM bank suffices.
- The softmax denominator lands in column 64 per lq-partition:
  normalization is reciprocal + per-partition tensor_scalar on DVE.
- Branches combine before the output projection (scalar_tensor_tensor),
  halving outproj; x returns to [hdk, lq] via PE transposes (host-fed
  identity matrix).
"""

import math
import os
from collections import deque
from contextlib import ExitStack

_DBG = os.environ.get("KV2_DEBUG") == "1"

import ml_dtypes
import numpy as np

import concourse.tile as tile
from concourse import bacc, mybir
from concourse import bass_utils

F32 = mybir.dt.float32
BF16 = mybir.dt.bfloat16
BF16_NP = ml_dtypes.bfloat16

B, L, D = 4, 2048, 512
H, DK = 8, 64
N_CORES = 8
HG = 4
DO = HG * DK
BN_EPS = 1e-5
NJ = D // 128
NLT = L // 128
WA, WB = 2048, 1024          # alternating staging widths
CA, CB = WA // 512, WB // 512  # chunks per tile: 4, 2
PAIR = CA + CB               # 6 chunks per A/B pair
NCHUNK = 512                 # 2p * 2br * 2hh * 4c * 16lkt / ... = 32 groups * 16

_cache = {}


def _build_program():
    nc = bacc.Bacc("TRN2", target_bir_lowering=False, debug=False,
                   num_devices=N_CORES)

    dt_in = {}
    for nm in ("xq", "xk", "xv"):
        dt_in[nm] = nc.dram_tensor(nm, [D, L], BF16, kind="ExternalInput").ap()
    for nm in ("wq", "wkg", "wv"):
        dt_in[nm] = nc.dram_tensor(nm, [D, DO], BF16, kind="ExternalInput").ap()
    dt_in["wk5"] = nc.dram_tensor("wk5", [5, D, DO], BF16, kind="ExternalInput").ap()
    dt_in["wo2"] = nc.dram_tensor("wo2", [128, 2, D], BF16, kind="ExternalInput").ap()
    dt_in["bkl"] = nc.dram_tensor("bkl", [DO], F32, kind="ExternalInput").ap()
    dt_in["ident"] = nc.dram_tensor("ident", [128, 128], BF16, kind="ExternalInput").ap()
    out_ap = nc.dram_tensor("out", [L, D], F32, kind="ExternalOutput").ap()

    with tile.TileContext(nc) as tc, ExitStack() as ctx:
        big = ctx.enter_context(tc.tile_pool(name="big", bufs=12))
        et = ctx.enter_context(tc.tile_pool(name="et", bufs=12))
        proj = ctx.enter_context(tc.tile_pool(name="projsb", bufs=1))
        norm = ctx.enter_context(tc.tile_pool(name="norm", bufs=8))
        ostage = ctx.enter_context(tc.tile_pool(name="ostage", bufs=3))
        stg = ctx.enter_context(tc.tile_pool(name="stg", bufs=1, space="PSUM"))
        wk = ctx.enter_context(tc.tile_pool(name="wk", bufs=1, space="PSUM"))

        # ---- persistent SBUF tensors ----
        wq_sb = proj.tile([128, NJ, DO], BF16, tag="wq")
        wk5_sb = proj.tile([128, 5, NJ, DO], BF16, tag="wk5")
        wkg_sb = proj.tile([128, NJ, DO], BF16, tag="wkg")
        wv_sb = proj.tile([128, NJ, DO], BF16, tag="wv")
        wo_sb = proj.tile([128, 2, D], BF16, tag="wo")
        id_sb = proj.tile([128, 128], BF16, tag="ident")
        bkl_sb = proj.tile([128, 2], F32, tag="bkl")
        qT_sb = proj.tile([128, 2, L], BF16, tag="qT")
        klT_sb = proj.tile([128, 2, L], BF16, tag="klT")
        kgT_sb = proj.tile([128, 2, L], BF16, tag="kgT")
        v_sb = proj.tile([128, NLT, HG, DK + 1], BF16, tag="v")
        x0_sb = proj.tile([128, 2, 2, 4, 4, DK], BF16, tag="x0")
        x1_sb = proj.tile([128, 2, 4, 4, DK], BF16, tag="x1")
        xT_sb = proj.tile([128, 2, NLT, 128], BF16, tag="xT")
        zw_sb = proj.tile([128, 512], BF16, tag="zw")

        # ---- input DMA: few big ops, 3 queues, need-by ordering ----
        LKP = L + 4
        kxall = big.tile([128, NJ, LKP], BF16, tag="kx", name="kxall", bufs=1)
        xqall = big.tile([128, NJ, L], BF16, tag="xq", name="xqall", bufs=1)
        xvall = big.tile([128, NJ, L], BF16, tag="xv", name="xvall", bufs=1)
        kx = [kxall[:, j, :] for j in range(NJ)]
        xq = [xqall[:, j, :] for j in range(NJ)]
        xv = [xvall[:, j, :] for j in range(NJ)]
        nc.vector.memset(kxall[:, :, 0:2], 0.0)
        nc.vector.memset(kxall[:, :, 2 + L:], 0.0)
        xk_r = dt_in["xk"].rearrange("(j p) l -> p j l", p=128)
        xq_r = dt_in["xq"].rearrange("(j p) l -> p j l", p=128)
        xv_r = dt_in["xv"].rearrange("(j p) l -> p j l", p=128)
        # The cost model serializes all DMA transfers on one resource, so
        # queue parallelism buys nothing: issue everything on one queue in
        # exact need order, sized so each lands just before its consumer.
        nc.sync.dma_start(kxall[:, :, 2:518], xk_r[:, :, 0:516])        # kgT qb0
        nc.sync.dma_start(wkg_sb[:], dt_in["wkg"].rearrange("(j p) o -> p j o", p=128))
        nc.sync.dma_start(wq_sb[:], dt_in["wq"].rearrange("(j p) o -> p j o", p=128))
        nc.sync.dma_start(xqall[:, :, 0:512], xq_r[:, :, 0:512])        # qT c0
        nc.sync.dma_start(kxall[:, :, 518:1034], xk_r[:, :, 516:1032])  # kgT qb1
        nc.sync.dma_start(kxall[:, :, 1034:2 + L], xk_r[:, :, 1032:])   # kgT qb2-3
        nc.sync.dma_start(xvall[:, 0:2, :], xv_r[:, 0:2, :])
        nc.sync.dma_start(wv_sb[:], dt_in["wv"].rearrange("(j p) o -> p j o", p=128))
        nc.sync.dma_start(xvall[:, 2:4, :], xv_r[:, 2:4, :])
        nc.sync.dma_start(xqall[:, :, 512:1024], xq_r[:, :, 512:1024])  # qT c1
        nc.sync.dma_start(wk5_sb[:], dt_in["wk5"].rearrange("t (j p) o -> p t j o", p=128))
        nc.sync.dma_start(xqall[:, :, 1024:], xq_r[:, :, 1024:])        # qT c2-3
        nc.sync.dma_start(bkl_sb[:], dt_in["bkl"].rearrange("(m p) -> p m", p=128))
        nc.sync.dma_start(wo_sb[:], dt_in["wo2"])
        nc.sync.dma_start(id_sb[:], dt_in["ident"])

        warm = proj.tile([1, 16], F32, tag="warmt")
        nc.vector.memset(warm[:], 0.0)
        nc.scalar.activation(warm[:], warm[:], mybir.ActivationFunctionType.Exp)

        # ---- PE warm-up: ramp the p-state while input DMA lands ----
        nc.vector.memset(zw_sb[:], 0.0)
        for i in range(12):
            zp = wk.tile([128, 512], F32, tag="pp", name=f"zp{i}")
            nc.tensor.matmul(zp[:], zw_sb[:, 0:128], zw_sb[:], start=True, stop=True)

        # ---- emitters ----
        def proj_chunk(dst_sb, w_sb, m, qb, src, bias=None, off=0):
            ps = wk.tile([128, 512], F32, tag="pp", name=f"pp{m}_{qb}")
            for j in range(NJ):
                nc.tensor.matmul(ps[:], w_sb[:, j, m * 128:(m + 1) * 128],
                                 src[j][:, off + qb * 512:off + qb * 512 + 512],
                                 start=(j == 0), stop=(j == NJ - 1))
            if bias is not None:
                nc.vector.tensor_scalar_add(
                    dst_sb[:, m, qb * 512:qb * 512 + 512], ps[:], bias[:, m:m + 1])
            else:
                nc.vector.tensor_copy(dst_sb[:, m, qb * 512:qb * 512 + 512], ps[:])

        def klT_burst(state, m, qb, t):
            if t == 0:
                state["ps"] = wk.tile([128, 512], F32, tag="pp", name=f"kl{m}_{qb}")
            ps = state["ps"]
            for j in range(NJ):
                nc.tensor.matmul(ps[:], wk5_sb[:, t, j, m * 128:(m + 1) * 128],
                                 kx[j][:, qb * 512 + t:qb * 512 + t + 512],
                                 start=(t == 0 and j == 0), stop=(t == 4 and j == NJ - 1))
            if t == 4:
                nc.vector.tensor_scalar_add(
                    klT_sb[:, m, qb * 512:qb * 512 + 512], ps[:], bkl_sb[:, m:m + 1])

        def v_chunk(lt):
            if lt == 0:
                nc.vector.memset(v_sb[:], 1.0)
            ps = wk.tile([128, 512], F32, tag="pp", name=f"vp{lt}")
            for j in range(NJ):
                nc.tensor.matmul(ps[:, :DO], xv[j][:, lt * 128:lt * 128 + 128],
                                 wv_sb[:, j, :],
                                 start=(j == 0), stop=(j == NJ - 1))
            nc.vector.tensor_copy(
                v_sb[:, lt, :, 0:DK],
                ps[:, :DO].rearrange("p (h d) -> p h d", h=HG))

        def av_sub(gi, sub):
            p, br, hh, c = GROUPS[gi]
            h = 2 * p + hh
            tag = ("pp" if (gi >= 28 and sub % 2 == 1) else "av")
            av = wk.tile([128, DK + 1], F32, tag=tag, name=f"av{gi}_{sub}")
            for lkt in range(NLT):
                e_t, slot = emap[gi * NLT + lkt]
                nc.tensor.matmul(
                    av[:],
                    e_t[:, slot * 512 + sub * 128:slot * 512 + sub * 128 + 128],
                    v_sb[:, lkt, h % HG, :],
                    start=(lkt == 0), stop=(lkt == NLT - 1))
            # one fast copy frees the PSUM bank; normalize from the copy so
            # the next av accumulation never waits on the norm round-trip
            avc = norm.tile([128, DK + 1], F32, tag="avc", name=f"avc{gi}_{sub}")
            nc.vector.tensor_copy(avc[:], av[:])
            rd = norm.tile([128, 1], F32, tag="rd", name=f"rd{gi}_{sub}")
            nc.vector.reciprocal(rd[:], avc[:, DK:DK + 1])
            if br == 0:
                nc.vector.tensor_scalar_mul(
                    x0_sb[:, p, hh, c, sub, :], avc[:, 0:DK], rd[:])
            else:
                nc.vector.scalar_tensor_tensor(
                    x1_sb[:, hh, c, sub, :], avc[:, 0:DK], rd[:],
                    x0_sb[:, p, hh, c, sub, :],
                    mybir.AluOpType.mult, mybir.AluOpType.add)

        def transp(p, c, sub):
            lt = c * 4 + sub
            tp = wk.tile([128, 128], BF16, tag="av", name=f"tp{p}_{lt}")
            for hh in range(2):
                nc.tensor.matmul(tp[hh * 64:hh * 64 + 64, :],
                                 x1_sb[:, hh, c, sub, :], id_sb[:],
                                 is_transpose=True)
            nc.vector.tensor_copy(xT_sb[:, p, lt, :], tp[:])

        def outproj(lt):
            po = wk.tile([128, 512], F32, tag=("pp" if lt % 2 == 0 else "av"),
                         name=f"po{lt}")
            for p in range(2):
                nc.tensor.matmul(po[:], xT_sb[:, p, lt, :], wo_sb[:, p, :],
                                 start=(p == 0), stop=(p == 1))
            ot = ostage.tile([128, D], F32, tag="ot", name=f"ot{lt}")
            nc.vector.tensor_copy(ot[:], po[:])
            nc.sync.dma_start(out_ap[lt * 128:lt * 128 + 128, :], ot[:])

        # ---- group sequence and chunk stream ----
        GROUPS = []
        for br in range(2):
            for p in range(2):
                for c in range(4):
                    for hh in range(2):
                        GROUPS.append((p, br, hh, c))

        emap = {}
        st = {"stg": None, "et": None, "w": 0}

        # quantum scheduler state
        HIQ = deque()   # (cycles, fn) latency-sensitive: av, transp, outproj
        LOQ = []        # (deadline_chunk, earliest_chunk, cycles, fn), dl-sorted
        credit = [0.0]
        turn = ["lo"]

        def lo_pop_ready(cid):
            for i, (dl, est, cyc, fn) in enumerate(LOQ):
                if est <= cid:
                    LOQ.pop(i)
                    return cyc, fn
                if dl > cid + 40:
                    break
            return None

        def pump(budget, cid):
            credit[0] = min(credit[0] + budget, 5000.0)
            nlo = nhi = 0
            while credit[0] > 0 and (HIQ or LOQ) and nlo + nhi < 4:
                did = False
                order = ("lo", "hi") if turn[0] == "lo" else ("hi", "lo")
                for pref in order:
                    if pref == "lo" and LOQ and nlo < 1:
                        got = lo_pop_ready(cid)
                        if got is None:
                            continue
                        cyc, fn = got
                        turn[0] = "hi"
                        nlo += 1
                    elif pref == "hi" and HIQ and nhi < 2:
                        cyc, fn = HIQ.popleft()
                        turn[0] = "lo"
                        nhi += 1
                    else:
                        continue
                    fn()
                    credit[0] -= cyc
                    did = True
                    break
                if not did:
                    break

        def force_due(cid):
            # at most one forced pop per chunk so score matmuls interleave
            # and cover the single-bank drain latency
            if LOQ and LOQ[0][0] <= cid:
                _, _, cyc, fn = LOQ.pop(0)
                fn()
                credit[0] -= cyc

        def emit_chunk(cid):
            gi, lkt = divmod(cid, NLT)
            p, br, hh, c = GROUPS[gi]
            kT = kgT_sb if br == 0 else klT_sb
            pos = cid % PAIR
            if pos == 0:
                st["stg"] = stg.tile([128, WA], F32, tag="sa", name=f"sa{cid}")
                st["et"] = et.tile([128, WA], BF16, tag="ea", name=f"ea{cid}")
                st["w"] = CA
            elif pos == CA:
                st["stg"] = stg.tile([128, WB], F32, tag="sb", name=f"sb{cid}")
                st["et"] = et.tile([128, WB], BF16, tag="eb", name=f"eb{cid}")
                st["w"] = CB
            slot = pos if pos < CA else pos - CA
            if _DBG:
                print(f"chunk {cid} (gi {gi} lkt {lkt})")
            force_due(cid)
            nc.tensor.matmul(
                st["stg"][:, slot * 512:slot * 512 + 512],
                kT[hh * 64:hh * 64 + 64, p, lkt * 128:lkt * 128 + 128],
                qT_sb[hh * 64:hh * 64 + 64, p, c * 512:c * 512 + 512],
                start=True, stop=True)
            emap[cid] = (st["et"], slot)
            if slot == st["w"] - 1 or cid == NCHUNK - 1:
                w = (slot + 1) * 512
                nc.scalar.activation(st["et"][:, :w], st["stg"][:, :w],
                                     mybir.ActivationFunctionType.Exp)
                act_cyc = (w * 0.8333 + 185.0) / 0.41666
                pump(act_cyc - w, cid)

        # ---- LOW queue: projections, (deadline, earliest) in chunk units ----
        import bisect

        def lo(cyc, fn, dl, est=0):
            if _DBG:
                fn0 = fn

                def fn(fn0=fn0, dl=dl):
                    print(f"  LO pop dl={dl} {fn0}")
                    fn0()
            bisect.insort(LOQ, (dl, est, cyc, fn), key=lambda x: x[0])

        for qb in range(1, 4):
            lo(2048, (lambda qb=qb: proj_chunk(kgT_sb, wkg_sb, 0, qb, kx, off=2)),
               4 * qb, 0)
        for lt in range(16):
            lo(1024, (lambda lt=lt: v_chunk(lt)), 14 + 2 * lt, 12)
        for qb in range(1, 4):
            lo(2048, (lambda qb=qb: proj_chunk(qT_sb, wq_sb, 0, qb, xq)),
               32 * qb - 6, 18 if qb == 1 else 28)
        for i in range(20):
            qb, t = divmod(i, 5)
            if t == 0:
                s = {}
            lo(2048, (lambda s=s, qb=qb, t=t: klT_burst(s, 0, qb, t)),
               30 + 10 * i, 24)
        for qb in range(4):
            lo(2048, (lambda qb=qb: proj_chunk(qT_sb, wq_sb, 1, qb, xq)),
               100 + 6 * qb, 30)
        for qb in range(4):
            lo(2048, (lambda qb=qb: proj_chunk(kgT_sb, wkg_sb, 1, qb, kx, off=2)),
               70 + 6 * qb, 16)
        for i in range(20):
            qb, t = divmod(i, 5)
            if t == 0:
                s2 = {}
            lo(2048, (lambda s2=s2, qb=qb, t=t: klT_burst(s2, 1, qb, t)),
               250 + 5 * i, 150)

        # HIGH pushes happen at group completion: precompute per-group
        def push_av_group(gi):
            for sub in range(4):
                def avfn(gi=gi, sub=sub):
                    if _DBG:
                        print(f"  HI pop av({gi},{sub})")
                    av_sub(gi, sub)
                HIQ.append((1040, avfn))
            p, br, hh, c = GROUPS[gi]
            if br == 1 and hh == 1:
                for sub in range(4):
                    HIQ.append((300, lambda p=p, c=c, sub=sub: transp(p, c, sub)))
                if p == 1:
                    for lt in range(c * 4, c * 4 + 4):
                        HIQ.append((1024, lambda lt=lt: outproj(lt)))

        # av push lag in groups: 3 early (v DMA must land first), then 1,
        # then 0 at the end so the tail chain is minimal
        def av_push_gi(gi):
            return gi + 3 if gi < 8 else (gi + 1 if gi < 28 else gi)

        # ---- prologue projections (rest of kgT m0 arrives via LOW DLs) ----
        proj_chunk(kgT_sb, wkg_sb, 0, 0, kx, off=2)
        proj_chunk(qT_sb, wq_sb, 0, 0, xq)

        # ---- main stream ----
        pushed = set()
        for cid in range(NCHUNK):
            emit_chunk(cid)
            if cid % NLT == NLT - 1:
                gdone = cid // NLT
                for g in range(32):
                    if g not in pushed and av_push_gi(g) <= gdone:
                        push_av_group(g)
                        pushed.add(g)

        # ---- tail ----
        for g in range(32):
            if g not in pushed:
                push_av_group(g)
                pushed.add(g)
        while HIQ or LOQ:
            if LOQ:
                _, _, _, fn = LOQ.pop(0)
            else:
                _, fn = HIQ.popleft()
            fn()

    nc.compile()
    return nc


def _host_prep(inputs):
    """Fold conv+bn+biases; build the 8 per-core input maps."""
    f32 = np.float32
    q = np.ascontiguousarray(inputs["query"], dtype=f32)
    k = np.ascontiguousarray(inputs["key"], dtype=f32)
    v = np.ascontiguousarray(inputs["value"], dtype=f32)
    w3 = np.asarray(inputs["conv_w3"], f32)
    w5 = np.asarray(inputs["conv_w5"], f32)
    b3 = np.asarray(inputs["conv_b3"], f32)
    b5 = np.asarray(inputs["conv_b5"], f32)
    gam = np.asarray(inputs["bn_gamma"], f32)
    bet = np.asarray(inputs["bn_beta"], f32)
    mu = np.asarray(inputs["bn_mean"], f32)
    var = np.asarray(inputs["bn_var"], f32)
    wq = np.asarray(inputs["wq"], f32)
    bq = np.asarray(inputs["bq"], f32)
    wkl = np.asarray(inputs["wkl"], f32)
    bkl = np.asarray(inputs["bkl"], f32)
    wkg = np.asarray(inputs["wkg"], f32)
    bkg = np.asarray(inputs["bkg"], f32)
    wv = np.asarray(inputs["wv"], f32)
    bv = np.asarray(inputs["bv"], f32)
    wo = np.asarray(inputs["wo"], f32)
    bo = np.asarray(inputs["bo"], f32)

    assert not np.any(bq) and not np.any(bkg), "nonzero q/kg bias unsupported"

    s_bn = gam / np.sqrt(var + BN_EPS)
    shift = np.concatenate([b3, b5]) * s_bn + (bet - mu * s_bn)
    wkl_s = wkl * s_bn[None, :]
    A3 = np.einsum("oc,cit->oit", wkl_s[:, :D], w3)
    A5 = np.einsum("oc,cit->oit", wkl_s[:, D:], w5)
    W5c = A5.copy()
    W5c[:, :, 1:4] += A3
    bkl_eff = wkl @ shift + bkl
    wq_eff = wq / math.sqrt(DK)
    bo_eff = bo + wo @ (2.0 * bv)

    bf = BF16_NP
    ident = np.eye(128, dtype=bf)
    in_maps = []
    for c in range(N_CORES):
        b = c // 2
        hg = c % 2
        sel = slice(hg * DO, hg * DO + DO)
        wo_sel = wo.T[sel, :]
        wo2 = np.ascontiguousarray(
            wo_sel.reshape(2, 2, 64, D).transpose(1, 2, 0, 3).reshape(128, 2, D))
        in_maps.append({
            "xq": np.ascontiguousarray(q[b].T).astype(bf),
            "xk": np.ascontiguousarray(k[b].T).astype(bf),
            "xv": np.ascontiguousarray(v[b].T).astype(bf),
            "wq": np.ascontiguousarray(wq_eff.T[:, sel]).astype(bf),
            "wk5": np.ascontiguousarray(W5c.transpose(2, 1, 0)[:, :, sel]).astype(bf),
            "wkg": np.ascontiguousarray(wkg.T[:, sel]).astype(bf),
            "wv": np.ascontiguousarray(wv.T[:, sel]).astype(bf),
            "wo2": wo2.astype(bf),
            "bkl": np.ascontiguousarray(bkl_eff[sel]).astype(f32),
            "ident": ident,
        })
    return in_maps, bo_eff


def kernel(**inputs) -> np.ndarray:
    if "nc" not in _cache:
        _cache["nc"] = _build_program()
    nc = _cache["nc"]
    in_maps, bo_eff = _host_prep(inputs)
    res = bass_utils.run_bass_kernel_spmd(
        nc, in_maps, core_ids=list(range(N_CORES)))
    out = np.zeros((B, L, D), np.float32)
    for c in range(N_CORES):
        out[c // 2] += res.results[c]["out"]
    out += bo_eff[None, None, :]
    return out


# revision 4
# speedup vs baseline: 1.0078x; 1.0078x over previous
"""Trainium2 Bass kernel v2 for nn_MultiHeadedAttention_4269197492266.

Same math as v1 (folded 5-tap conv local-key path, batch x head-group
sharding, ones-column softmax denominator), restructured around the
cost model:

- Score chunks stream continuously into [128,1536] PSUM staging tiles
  (two 3-bank buffers, ping-pong); exp runs as 171 wide ACT instructions (~250us, the
  ACT floor for 33.5M exps/core).  All other PE work (projections, conv
  taps, AV, transposes, outproj) is queued as sub-microsecond quanta and
  popped between staging tiles under a cycle budget with emission
  deadlines, so the in-order PE queue never starves ACT.
- AV is reoriented: out[lq, dk+1] = e_chunk^T @ V -- each matmul streams
  65 columns instead of 512, halving AV cost.  Group-major score order
  makes AV accumulators sequential: one PSUM bank suffices.
- The softmax denominator lands in column 64 per lq-partition:
  normalization is reciprocal + per-partition tensor_scalar on DVE.
- Branches combine before the output projection (scalar_tensor_tensor),
  halving outproj; x returns to [hdk, lq] via PE transposes (host-fed
  identity matrix).
"""

import math
import os
from collections import deque
from contextlib import ExitStack

_DBG = os.environ.get("KV2_DEBUG") == "1"

import ml_dtypes
import numpy as np

import concourse.tile as tile
from concourse import bacc, mybir
from concourse import bass_utils

F32 = mybir.dt.float32
BF16 = mybir.dt.bfloat16
BF16_NP = ml_dtypes.bfloat16

B, L, D = 4, 2048, 512
H, DK = 8, 64
N_CORES = 8
HG = 4
DO = HG * DK
BN_EPS = 1e-5
NJ = D // 128
NLT = L // 128
WA, WB = 1536, 1536          # alternating staging widths
CA, CB = WA // 512, WB // 512  # chunks per tile: 4, 2
PAIR = CA + CB               # 6 chunks per A/B pair
NCHUNK = 512                 # 2p * 2br * 2hh * 4c * 16lkt / ... = 32 groups * 16

_cache = {}


def _build_program():
    nc = bacc.Bacc("TRN2", target_bir_lowering=False, debug=False,
                   num_devices=N_CORES)

    dt_in = {}
    for nm in ("xq", "xk", "xv"):
        dt_in[nm] = nc.dram_tensor(nm, [D, L], BF16, kind="ExternalInput").ap()
    for nm in ("wq", "wkg", "wv"):
        dt_in[nm] = nc.dram_tensor(nm, [D, DO], BF16, kind="ExternalInput").ap()
    dt_in["wk5"] = nc.dram_tensor("wk5", [5, D, DO], BF16, kind="ExternalInput").ap()
    dt_in["wo2"] = nc.dram_tensor("wo2", [128, 2, D], BF16, kind="ExternalInput").ap()
    dt_in["bkl"] = nc.dram_tensor("bkl", [DO], F32, kind="ExternalInput").ap()
    dt_in["ident"] = nc.dram_tensor("ident", [128, 128], BF16, kind="ExternalInput").ap()
    out_ap = nc.dram_tensor("out", [L, D], F32, kind="ExternalOutput").ap()

    with tile.TileContext(nc) as tc, ExitStack() as ctx:
        big = ctx.enter_context(tc.tile_pool(name="big", bufs=12))
        et = ctx.enter_context(tc.tile_pool(name="et", bufs=12))
        proj = ctx.enter_context(tc.tile_pool(name="projsb", bufs=1))
        norm = ctx.enter_context(tc.tile_pool(name="norm", bufs=8))
        ostage = ctx.enter_context(tc.tile_pool(name="ostage", bufs=3))
        stg = ctx.enter_context(tc.tile_pool(name="stg", bufs=1, space="PSUM"))
        wk = ctx.enter_context(tc.tile_pool(name="wk", bufs=1, space="PSUM"))

        # ---- persistent SBUF tensors ----
        wq_sb = proj.tile([128, NJ, DO], BF16, tag="wq")
        wk5_sb = proj.tile([128, 5, NJ, DO], BF16, tag="wk5")
        wkg_sb = proj.tile([128, NJ, DO], BF16, tag="wkg")
        wv_sb = proj.tile([128, NJ, DO], BF16, tag="wv")
        wo_sb = proj.tile([128, 2, D], BF16, tag="wo")
        id_sb = proj.tile([128, 128], BF16, tag="ident")
        bkl_sb = proj.tile([128, 2], F32, tag="bkl")
        qT_sb = proj.tile([128, 2, L], BF16, tag="qT")
        klT_sb = proj.tile([128, 2, L], BF16, tag="klT")
        kgT_sb = proj.tile([128, 2, L], BF16, tag="kgT")
        v_sb = proj.tile([128, NLT, HG, DK + 1], BF16, tag="v")
        x0_sb = proj.tile([128, 2, 2, 4, 4, DK], BF16, tag="x0")
        x1_sb = proj.tile([128, 2, 4, 4, DK], BF16, tag="x1")
        xT_sb = proj.tile([128, 2, NLT, 128], BF16, tag="xT")
        zw_sb = proj.tile([128, 512], BF16, tag="zw")

        # ---- input DMA: few big ops, 3 queues, need-by ordering ----
        LKP = L + 4
        kxall = big.tile([128, NJ, LKP], BF16, tag="kx", name="kxall", bufs=1)
        xqall = big.tile([128, NJ, L], BF16, tag="xq", name="xqall", bufs=1)
        xvall = big.tile([128, NJ, L], BF16, tag="xv", name="xvall", bufs=1)
        kx = [kxall[:, j, :] for j in range(NJ)]
        xq = [xqall[:, j, :] for j in range(NJ)]
        xv = [xvall[:, j, :] for j in range(NJ)]
        nc.vector.memset(kxall[:, :, 0:2], 0.0)
        nc.vector.memset(kxall[:, :, 2 + L:], 0.0)
        xk_r = dt_in["xk"].rearrange("(j p) l -> p j l", p=128)
        xq_r = dt_in["xq"].rearrange("(j p) l -> p j l", p=128)
        xv_r = dt_in["xv"].rearrange("(j p) l -> p j l", p=128)
        # The cost model serializes all DMA transfers on one resource, so
        # queue parallelism buys nothing: issue everything on one queue in
        # exact need order, sized so each lands just before its consumer.
        nc.sync.dma_start(kxall[:, :, 2:518], xk_r[:, :, 0:516])        # kgT qb0
        nc.sync.dma_start(wkg_sb[:], dt_in["wkg"].rearrange("(j p) o -> p j o", p=128))
        nc.sync.dma_start(wq_sb[:], dt_in["wq"].rearrange("(j p) o -> p j o", p=128))
        nc.sync.dma_start(xqall[:, :, 0:512], xq_r[:, :, 0:512])        # qT c0
        nc.sync.dma_start(kxall[:, :, 518:1034], xk_r[:, :, 516:1032])  # kgT qb1
        nc.sync.dma_start(kxall[:, :, 1034:2 + L], xk_r[:, :, 1032:])   # kgT qb2-3
        nc.sync.dma_start(xvall[:, 0:2, :], xv_r[:, 0:2, :])
        nc.sync.dma_start(wv_sb[:], dt_in["wv"].rearrange("(j p) o -> p j o", p=128))
        nc.sync.dma_start(xvall[:, 2:4, :], xv_r[:, 2:4, :])
        nc.sync.dma_start(xqall[:, :, 512:1024], xq_r[:, :, 512:1024])  # qT c1
        nc.sync.dma_start(wk5_sb[:], dt_in["wk5"].rearrange("t (j p) o -> p t j o", p=128))
        nc.sync.dma_start(xqall[:, :, 1024:], xq_r[:, :, 1024:])        # qT c2-3
        nc.sync.dma_start(bkl_sb[:], dt_in["bkl"].rearrange("(m p) -> p m", p=128))
        nc.sync.dma_start(wo_sb[:], dt_in["wo2"])
        nc.sync.dma_start(id_sb[:], dt_in["ident"])

        warm = proj.tile([1, 16], F32, tag="warmt")
        nc.vector.memset(warm[:], 0.0)
        nc.scalar.activation(warm[:], warm[:], mybir.ActivationFunctionType.Exp)

        # ---- PE warm-up: ramp the p-state while input DMA lands ----
        nc.vector.memset(zw_sb[:], 0.0)
        for i in range(12):
            zp = wk.tile([128, 512], F32, tag="pp", name=f"zp{i}")
            nc.tensor.matmul(zp[:], zw_sb[:, 0:128], zw_sb[:], start=True, stop=True)

        # ---- emitters ----
        def proj_chunk(dst_sb, w_sb, m, qb, src, bias=None, off=0):
            ps = wk.tile([128, 512], F32, tag="pp", name=f"pp{m}_{qb}")
            for j in range(NJ):
                nc.tensor.matmul(ps[:], w_sb[:, j, m * 128:(m + 1) * 128],
                                 src[j][:, off + qb * 512:off + qb * 512 + 512],
                                 start=(j == 0), stop=(j == NJ - 1))
            if bias is not None:
                nc.vector.tensor_scalar_add(
                    dst_sb[:, m, qb * 512:qb * 512 + 512], ps[:], bias[:, m:m + 1])
            else:
                nc.vector.tensor_copy(dst_sb[:, m, qb * 512:qb * 512 + 512], ps[:])

        def klT_burst(state, m, qb, t):
            if t == 0:
                state["ps"] = wk.tile([128, 512], F32, tag="pp", name=f"kl{m}_{qb}")
            ps = state["ps"]
            for j in range(NJ):
                nc.tensor.matmul(ps[:], wk5_sb[:, t, j, m * 128:(m + 1) * 128],
                                 kx[j][:, qb * 512 + t:qb * 512 + t + 512],
                                 start=(t == 0 and j == 0), stop=(t == 4 and j == NJ - 1))
            if t == 4:
                nc.vector.tensor_scalar_add(
                    klT_sb[:, m, qb * 512:qb * 512 + 512], ps[:], bkl_sb[:, m:m + 1])

        def v_chunk(lt):
            if lt == 0:
                nc.vector.memset(v_sb[:], 1.0)
            ps = wk.tile([128, 512], F32, tag="pp", name=f"vp{lt}")
            for j in range(NJ):
                nc.tensor.matmul(ps[:, :DO], xv[j][:, lt * 128:lt * 128 + 128],
                                 wv_sb[:, j, :],
                                 start=(j == 0), stop=(j == NJ - 1))
            nc.vector.tensor_copy(
                v_sb[:, lt, :, 0:DK],
                ps[:, :DO].rearrange("p (h d) -> p h d", h=HG))

        def av_sub(gi, sub):
            p, br, hh, c = GROUPS[gi]
            h = 2 * p + hh
            tag = ("pp" if (gi >= 28 and sub % 2 == 1) else "av")
            av = wk.tile([128, DK + 1], F32, tag=tag, name=f"av{gi}_{sub}")
            for lkt in range(NLT):
                e_t, slot = emap[gi * NLT + lkt]
                nc.tensor.matmul(
                    av[:],
                    e_t[:, slot * 512 + sub * 128:slot * 512 + sub * 128 + 128],
                    v_sb[:, lkt, h % HG, :],
                    start=(lkt == 0), stop=(lkt == NLT - 1))
            # one fast copy frees the PSUM bank; normalize from the copy so
            # the next av accumulation never waits on the norm round-trip
            avc = norm.tile([128, DK + 1], F32, tag="avc", name=f"avc{gi}_{sub}")
            nc.vector.tensor_copy(avc[:], av[:])
            rd = norm.tile([128, 1], F32, tag="rd", name=f"rd{gi}_{sub}")
            nc.vector.reciprocal(rd[:], avc[:, DK:DK + 1])
            if br == 0:
                nc.vector.tensor_scalar_mul(
                    x0_sb[:, p, hh, c, sub, :], avc[:, 0:DK], rd[:])
            else:
                nc.vector.scalar_tensor_tensor(
                    x1_sb[:, hh, c, sub, :], avc[:, 0:DK], rd[:],
                    x0_sb[:, p, hh, c, sub, :],
                    mybir.AluOpType.mult, mybir.AluOpType.add)

        def transp(p, c, sub):
            lt = c * 4 + sub
            tp = wk.tile([128, 128], BF16, tag="av", name=f"tp{p}_{lt}")
            for hh in range(2):
                nc.tensor.matmul(tp[hh * 64:hh * 64 + 64, :],
                                 x1_sb[:, hh, c, sub, :], id_sb[:],
                                 is_transpose=True)
            nc.vector.tensor_copy(xT_sb[:, p, lt, :], tp[:])

        def outproj(lt):
            po = wk.tile([128, 512], F32, tag=("pp" if lt % 2 == 0 else "av"),
                         name=f"po{lt}")
            for p in range(2):
                nc.tensor.matmul(po[:], xT_sb[:, p, lt, :], wo_sb[:, p, :],
                                 start=(p == 0), stop=(p == 1))
            ot = ostage.tile([128, D], F32, tag="ot", name=f"ot{lt}")
            nc.vector.tensor_copy(ot[:], po[:])
            nc.sync.dma_start(out_ap[lt * 128:lt * 128 + 128, :], ot[:])

        # ---- group sequence and chunk stream ----
        GROUPS = []
        for br in range(2):
            for p in range(2):
                for c in range(4):
                    for hh in range(2):
                        GROUPS.append((p, br, hh, c))

        emap = {}
        st = {"stg": None, "et": None, "w": 0}

        # quantum scheduler state
        HIQ = deque()   # (cycles, fn) latency-sensitive: av, transp, outproj
        LOQ = []        # (deadline_chunk, earliest_chunk, cycles, fn), dl-sorted
        credit = [0.0]
        turn = ["lo"]

        def lo_pop_ready(cid):
            for i, (dl, est, cyc, fn) in enumerate(LOQ):
                if est <= cid:
                    LOQ.pop(i)
                    return cyc, fn
                if dl > cid + 40:
                    break
            return None

        def pump(budget, cid):
            credit[0] = min(credit[0] + budget, 5000.0)
            nlo = nhi = 0
            while credit[0] > 0 and (HIQ or LOQ) and nlo + nhi < 4:
                did = False
                order = ("lo", "hi") if turn[0] == "lo" else ("hi", "lo")
                for pref in order:
                    if pref == "lo" and LOQ and nlo < 1:
                        got = lo_pop_ready(cid)
                        if got is None:
                            continue
                        cyc, fn = got
                        turn[0] = "hi"
                        nlo += 1
                    elif pref == "hi" and HIQ and nhi < 2:
                        cyc, fn = HIQ.popleft()
                        turn[0] = "lo"
                        nhi += 1
                    else:
                        continue
                    fn()
                    credit[0] -= cyc
                    did = True
                    break
                if not did:
                    break

        def force_due(cid):
            # at most one forced pop per chunk so score matmuls interleave
            # and cover the single-bank drain latency
            if LOQ and LOQ[0][0] <= cid:
                _, _, cyc, fn = LOQ.pop(0)
                fn()
                credit[0] -= cyc

        def emit_chunk(cid):
            gi, lkt = divmod(cid, NLT)
            p, br, hh, c = GROUPS[gi]
            kT = kgT_sb if br == 0 else klT_sb
            pos = cid % PAIR
            if pos == 0:
                st["stg"] = stg.tile([128, WA], F32, tag="sa", name=f"sa{cid}")
                st["et"] = et.tile([128, WA], BF16, tag="ea", name=f"ea{cid}")
                st["w"] = CA
            elif pos == CA:
                st["stg"] = stg.tile([128, WB], F32, tag="sb", name=f"sb{cid}")
                st["et"] = et.tile([128, WB], BF16, tag="eb", name=f"eb{cid}")
                st["w"] = CB
            slot = pos if pos < CA else pos - CA
            if _DBG:
                print(f"chunk {cid} (gi {gi} lkt {lkt})")
            force_due(cid)
            nc.tensor.matmul(
                st["stg"][:, slot * 512:slot * 512 + 512],
                kT[hh * 64:hh * 64 + 64, p, lkt * 128:lkt * 128 + 128],
                qT_sb[hh * 64:hh * 64 + 64, p, c * 512:c * 512 + 512],
                start=True, stop=True)
            emap[cid] = (st["et"], slot)
            if slot == st["w"] - 1 or cid == NCHUNK - 1:
                w = (slot + 1) * 512
                nc.scalar.activation(st["et"][:, :w], st["stg"][:, :w],
                                     mybir.ActivationFunctionType.Exp)
                act_cyc = (w * 0.8333 + 185.0) / 0.41666
                pump(act_cyc - w, cid)

        # ---- LOW queue: projections, (deadline, earliest) in chunk units ----
        import bisect

        def lo(cyc, fn, dl, est=0):
            if _DBG:
                fn0 = fn

                def fn(fn0=fn0, dl=dl):
                    print(f"  LO pop dl={dl} {fn0}")
                    fn0()
            bisect.insort(LOQ, (dl, est, cyc, fn), key=lambda x: x[0])

        for qb in range(1, 4):
            lo(2048, (lambda qb=qb: proj_chunk(kgT_sb, wkg_sb, 0, qb, kx, off=2)),
               4 * qb, 0)
        for lt in range(16):
            lo(1024, (lambda lt=lt: v_chunk(lt)), 14 + 2 * lt, 12)
        for qb in range(1, 4):
            lo(2048, (lambda qb=qb: proj_chunk(qT_sb, wq_sb, 0, qb, xq)),
               32 * qb - 6, 18 if qb == 1 else 28)
        for i in range(20):
            qb, t = divmod(i, 5)
            if t == 0:
                s = {}
            lo(2048, (lambda s=s, qb=qb, t=t: klT_burst(s, 0, qb, t)),
               30 + 10 * i, 24)
        for qb in range(4):
            lo(2048, (lambda qb=qb: proj_chunk(qT_sb, wq_sb, 1, qb, xq)),
               100 + 6 * qb, 30)
        for qb in range(4):
            lo(2048, (lambda qb=qb: proj_chunk(kgT_sb, wkg_sb, 1, qb, kx, off=2)),
               70 + 6 * qb, 16)
        for i in range(20):
            qb, t = divmod(i, 5)
            if t == 0:
                s2 = {}
            lo(2048, (lambda s2=s2, qb=qb, t=t: klT_burst(s2, 1, qb, t)),
               250 + 5 * i, 150)

        # HIGH pushes happen at group completion: precompute per-group
        def push_av_group(gi):
            for sub in range(4):
                def avfn(gi=gi, sub=sub):
                    if _DBG:
                        print(f"  HI pop av({gi},{sub})")
                    av_sub(gi, sub)
                HIQ.append((1040, avfn))
            p, br, hh, c = GROUPS[gi]
            if br == 1 and hh == 1:
                for sub in range(4):
                    HIQ.append((300, lambda p=p, c=c, sub=sub: transp(p, c, sub)))
                if p == 1:
                    for lt in range(c * 4, c * 4 + 4):
                        HIQ.append((1024, lambda lt=lt: outproj(lt)))

        # av push lag in groups: 3 early (v DMA must land first), then 1,
        # then 0 at the end so the tail chain is minimal
        def av_push_gi(gi):
            return gi + 3 if gi < 8 else (gi + 1 if gi < 28 else gi)

        # ---- prologue projections (rest of kgT m0 arrives via LOW DLs) ----
        proj_chunk(kgT_sb, wkg_sb, 0, 0, kx, off=2)
        proj_chunk(qT_sb, wq_sb, 0, 0, xq)

        # ---- main stream ----
        pushed = set()
        for cid in range(NCHUNK):
            emit_chunk(cid)
            if cid % NLT == NLT - 1:
                gdone = cid // NLT
                for g in range(32):
                    if g not in pushed and av_push_gi(g) <= gdone:
                        push_av_group(g)
                        pushed.add(g)

        # ---- tail ----
        for g in range(32):
            if g not in pushed:
                push_av_group(g)
                pushed.add(g)
        while HIQ or LOQ:
            if LOQ:
                _, _, _, fn = LOQ.pop(0)
            else:
                _, fn = HIQ.popleft()
            fn()

    nc.compile()
    return nc


def _host_prep(inputs):
    """Fold conv+bn+biases; build the 8 per-core input maps."""
    f32 = np.float32
    q = np.ascontiguousarray(inputs["query"], dtype=f32)
    k = np.ascontiguousarray(inputs["key"], dtype=f32)
    v = np.ascontiguousarray(inputs["value"], dtype=f32)
    w3 = np.asarray(inputs["conv_w3"], f32)
    w5 = np.asarray(inputs["conv_w5"], f32)
    b3 = np.asarray(inputs["conv_b3"], f32)
    b5 = np.asarray(inputs["conv_b5"], f32)
    gam = np.asarray(inputs["bn_gamma"], f32)
    bet = np.asarray(inputs["bn_beta"], f32)
    mu = np.asarray(inputs["bn_mean"], f32)
    var = np.asarray(inputs["bn_var"], f32)
    wq = np.asarray(inputs["wq"], f32)
    bq = np.asarray(inputs["bq"], f32)
    wkl = np.asarray(inputs["wkl"], f32)
    bkl = np.asarray(inputs["bkl"], f32)
    wkg = np.asarray(inputs["wkg"], f32)
    bkg = np.asarray(inputs["bkg"], f32)
    wv = np.asarray(inputs["wv"], f32)
    bv = np.asarray(inputs["bv"], f32)
    wo = np.asarray(inputs["wo"], f32)
    bo = np.asarray(inputs["bo"], f32)

    assert not np.any(bq) and not np.any(bkg), "nonzero q/kg bias unsupported"

    s_bn = gam / np.sqrt(var + BN_EPS)
    shift = np.concatenate([b3, b5]) * s_bn + (bet - mu * s_bn)
    wkl_s = wkl * s_bn[None, :]
    A3 = np.einsum("oc,cit->oit", wkl_s[:, :D], w3)
    A5 = np.einsum("oc,cit->oit", wkl_s[:, D:], w5)
    W5c = A5.copy()
    W5c[:, :, 1:4] += A3
    bkl_eff = wkl @ shift + bkl
    wq_eff = wq / math.sqrt(DK)
    bo_eff = bo + wo @ (2.0 * bv)

    bf = BF16_NP
    ident = np.eye(128, dtype=bf)
    in_maps = []
    for c in range(N_CORES):
        b = c // 2
        hg = c % 2
        sel = slice(hg * DO, hg * DO + DO)
        wo_sel = wo.T[sel, :]
        wo2 = np.ascontiguousarray(
            wo_sel.reshape(2, 2, 64, D).transpose(1, 2, 0, 3).reshape(128, 2, D))
        in_maps.append({
            "xq": np.ascontiguousarray(q[b].T).astype(bf),
            "xk": np.ascontiguousarray(k[b].T).astype(bf),
            "xv": np.ascontiguousarray(v[b].T).astype(bf),
            "wq": np.ascontiguousarray(wq_eff.T[:, sel]).astype(bf),
            "wk5": np.ascontiguousarray(W5c.transpose(2, 1, 0)[:, :, sel]).astype(bf),
            "wkg": np.ascontiguousarray(wkg.T[:, sel]).astype(bf),
            "wv": np.ascontiguousarray(wv.T[:, sel]).astype(bf),
            "wo2": wo2.astype(bf),
            "bkl": np.ascontiguousarray(bkl_eff[sel]).astype(f32),
            "ident": ident,
        })
    return in_maps, bo_eff


def kernel(**inputs) -> np.ndarray:
    if "nc" not in _cache:
        _cache["nc"] = _build_program()
    nc = _cache["nc"]
    in_maps, bo_eff = _host_prep(inputs)
    res = bass_utils.run_bass_kernel_spmd(
        nc, in_maps, core_ids=list(range(N_CORES)))
    out = np.zeros((B, L, D), np.float32)
    for c in range(N_CORES):
        out[c // 2] += res.results[c]["out"]
    out += bo_eff[None, None, :]
    return out


# revision 6
# speedup vs baseline: 1.0505x; 1.0424x over previous
"""Trainium2 Bass kernel v2 for nn_MultiHeadedAttention_4269197492266.

Same math as v1 (folded 5-tap conv local-key path, batch x head-group
sharding, ones-column softmax denominator), restructured around the
cost model:

- Score chunks stream continuously into [128,1536] PSUM staging tiles
  (two 3-bank buffers, ping-pong); exp runs as 171 wide ACT instructions (~250us, the
  ACT floor for 33.5M exps/core).  All other PE work (projections, conv
  taps, AV, transposes, outproj) is queued as sub-microsecond quanta and
  popped between staging tiles under a cycle budget with emission
  deadlines, so the in-order PE queue never starves ACT.
- AV is reoriented: out[lq, dk+1] = e_chunk^T @ V -- each matmul streams
  65 columns instead of 512, halving AV cost.  Group-major score order
  makes AV accumulators sequential: one PSUM bank suffices.
- The softmax denominator lands in column 64 per lq-partition:
  normalization is reciprocal + per-partition tensor_scalar on DVE.
- Branches combine before the output projection (scalar_tensor_tensor),
  halving outproj; x returns to [hdk, lq] via PE transposes (host-fed
  identity matrix).
"""

import math
import os
from collections import deque
from contextlib import ExitStack

_DBG = os.environ.get("KV2_DEBUG") == "1"

import ml_dtypes
import numpy as np

import concourse.tile as tile
from concourse import bacc, mybir
from concourse import bass_utils

F32 = mybir.dt.float32
BF16 = mybir.dt.bfloat16
BF16_NP = ml_dtypes.bfloat16

B, L, D = 4, 2048, 512
H, DK = 8, 64
N_CORES = 8
HG = 4
DO = HG * DK
BN_EPS = 1e-5
NJ = D // 128
NLT = L // 128
WA, WB = 1536, 1536          # alternating staging widths
CA, CB = WA // 512, WB // 512  # chunks per tile: 4, 2
PAIR = CA + CB               # 6 chunks per A/B pair
NCHUNK = 512                 # 2p * 2br * 2hh * 4c * 16lkt / ... = 32 groups * 16

_cache = {}


def _build_program():
    nc = bacc.Bacc("TRN2", target_bir_lowering=False, debug=False,
                   num_devices=N_CORES)

    dt_in = {}
    for nm in ("xq", "xk", "xv"):
        dt_in[nm] = nc.dram_tensor(nm, [D, L], BF16, kind="ExternalInput").ap()
    for nm in ("wq", "wkg", "wv"):
        dt_in[nm] = nc.dram_tensor(nm, [D, DO], BF16, kind="ExternalInput").ap()
    dt_in["wk5"] = nc.dram_tensor("wk5", [5, D, DO], BF16, kind="ExternalInput").ap()
    dt_in["wo2"] = nc.dram_tensor("wo2", [128, 2, D], BF16, kind="ExternalInput").ap()
    dt_in["bkl"] = nc.dram_tensor("bkl", [DO], F32, kind="ExternalInput").ap()
    dt_in["ident"] = nc.dram_tensor("ident", [128, 128], BF16, kind="ExternalInput").ap()
    out_ap = nc.dram_tensor("out", [L, D], F32, kind="ExternalOutput").ap()

    with tile.TileContext(nc) as tc, ExitStack() as ctx:
        big = ctx.enter_context(tc.tile_pool(name="big", bufs=12))
        et = ctx.enter_context(tc.tile_pool(name="et", bufs=12))
        proj = ctx.enter_context(tc.tile_pool(name="projsb", bufs=1))
        norm = ctx.enter_context(tc.tile_pool(name="norm", bufs=8))
        ostage = ctx.enter_context(tc.tile_pool(name="ostage", bufs=3))
        stg = ctx.enter_context(tc.tile_pool(name="stg", bufs=1, space="PSUM"))
        wk = ctx.enter_context(tc.tile_pool(name="wk", bufs=1, space="PSUM"))

        # ---- persistent SBUF tensors ----
        wq_sb = proj.tile([128, NJ, DO], BF16, tag="wq")
        wk5_sb = proj.tile([128, 5, NJ, DO], BF16, tag="wk5")
        wkg_sb = proj.tile([128, NJ, DO], BF16, tag="wkg")
        wv_sb = proj.tile([128, NJ, DO], BF16, tag="wv")
        wo_sb = proj.tile([128, 2, D], BF16, tag="wo")
        id_sb = proj.tile([128, 128], BF16, tag="ident")
        bkl_sb = proj.tile([128, 2], F32, tag="bkl")
        qT_sb = proj.tile([128, 2, L], BF16, tag="qT")
        klT_sb = proj.tile([128, 2, L], BF16, tag="klT")
        kgT_sb = proj.tile([128, 2, L], BF16, tag="kgT")
        v_sb = proj.tile([128, NLT, HG, DK + 1], BF16, tag="v")
        x0_sb = proj.tile([128, 2, 2, 4, 4, DK], BF16, tag="x0")
        x1_sb = proj.tile([128, 2, 4, 4, DK], BF16, tag="x1")
        xT_sb = proj.tile([128, 2, NLT, 128], BF16, tag="xT")
        zw_sb = proj.tile([128, 512], BF16, tag="zw")

        # ---- input DMA: few big ops, 3 queues, need-by ordering ----
        LKP = L + 4
        kxall = big.tile([128, NJ, LKP], BF16, tag="kx", name="kxall", bufs=1)
        xqall = big.tile([128, NJ, L], BF16, tag="xq", name="xqall", bufs=1)
        xvall = big.tile([128, NJ, L], BF16, tag="xv", name="xvall", bufs=1)
        kx = [kxall[:, j, :] for j in range(NJ)]
        xq = [xqall[:, j, :] for j in range(NJ)]
        xv = [xvall[:, j, :] for j in range(NJ)]
        nc.vector.memset(kxall[:, :, 0:2], 0.0)
        nc.vector.memset(kxall[:, :, 2 + L:], 0.0)
        xk_r = dt_in["xk"].rearrange("(j p) l -> p j l", p=128)
        xq_r = dt_in["xq"].rearrange("(j p) l -> p j l", p=128)
        xv_r = dt_in["xv"].rearrange("(j p) l -> p j l", p=128)
        # The cost model serializes all DMA transfers on one resource, so
        # queue parallelism buys nothing: issue everything on one queue in
        # exact need order, sized so each lands just before its consumer.
        nc.sync.dma_start(kxall[:, :, 2:518], xk_r[:, :, 0:516])        # kgT qb0
        nc.sync.dma_start(wkg_sb[:], dt_in["wkg"].rearrange("(j p) o -> p j o", p=128))
        nc.sync.dma_start(wq_sb[:], dt_in["wq"].rearrange("(j p) o -> p j o", p=128))
        nc.sync.dma_start(xqall[:, :, 0:512], xq_r[:, :, 0:512])        # qT c0
        nc.sync.dma_start(kxall[:, :, 518:1034], xk_r[:, :, 516:1032])  # kgT qb1
        nc.sync.dma_start(kxall[:, :, 1034:2 + L], xk_r[:, :, 1032:])   # kgT qb2-3
        nc.sync.dma_start(xvall[:, 0:2, :], xv_r[:, 0:2, :])
        nc.sync.dma_start(wv_sb[:], dt_in["wv"].rearrange("(j p) o -> p j o", p=128))
        nc.sync.dma_start(xvall[:, 2:4, :], xv_r[:, 2:4, :])
        nc.sync.dma_start(xqall[:, :, 512:1024], xq_r[:, :, 512:1024])  # qT c1
        nc.sync.dma_start(wk5_sb[:], dt_in["wk5"].rearrange("t (j p) o -> p t j o", p=128))
        nc.sync.dma_start(xqall[:, :, 1024:], xq_r[:, :, 1024:])        # qT c2-3
        nc.sync.dma_start(bkl_sb[:], dt_in["bkl"].rearrange("(m p) -> p m", p=128))
        nc.sync.dma_start(wo_sb[:], dt_in["wo2"])
        nc.sync.dma_start(id_sb[:], dt_in["ident"])

        warm = proj.tile([1, 16], F32, tag="warmt")
        nc.vector.memset(warm[:], 0.0)
        nc.scalar.activation(warm[:], warm[:], mybir.ActivationFunctionType.Exp)

        # ---- PE warm-up: ramp the p-state while input DMA lands ----
        nc.vector.memset(zw_sb[:], 0.0)
        for i in range(12):
            zp = wk.tile([128, 512], F32, tag="pp", name=f"zp{i}")
            nc.tensor.matmul(zp[:], zw_sb[:, 0:128], zw_sb[:], start=True, stop=True)

        # ---- emitters ----
        def proj_chunk(dst_sb, w_sb, m, qb, src, bias=None, off=0, ptag="pp"):
            ps = wk.tile([128, 512], F32, tag=ptag, name=f"pp{m}_{qb}")
            for j in range(NJ):
                nc.tensor.matmul(ps[:], w_sb[:, j, m * 128:(m + 1) * 128],
                                 src[j][:, off + qb * 512:off + qb * 512 + 512],
                                 start=(j == 0), stop=(j == NJ - 1))
            if bias is not None:
                nc.vector.tensor_scalar_add(
                    dst_sb[:, m, qb * 512:qb * 512 + 512], ps[:], bias[:, m:m + 1])
            else:
                nc.vector.tensor_copy(dst_sb[:, m, qb * 512:qb * 512 + 512], ps[:])

        def klT_burst(state, m, qb, t):
            if t == 0:
                tg = "av" if (m == 0 and qb == 0) else "pp"
                state["ps"] = wk.tile([128, 512], F32, tag=tg, name=f"kl{m}_{qb}")
            ps = state["ps"]
            for j in range(NJ):
                nc.tensor.matmul(ps[:], wk5_sb[:, t, j, m * 128:(m + 1) * 128],
                                 kx[j][:, qb * 512 + t:qb * 512 + t + 512],
                                 start=(t == 0 and j == 0), stop=(t == 4 and j == NJ - 1))
            if t == 4:
                nc.vector.tensor_scalar_add(
                    klT_sb[:, m, qb * 512:qb * 512 + 512], ps[:], bkl_sb[:, m:m + 1])

        def v_chunk(lt):
            if lt == 0:
                nc.vector.memset(v_sb[:], 1.0)
            ps = wk.tile([128, 512], F32, tag=("pp" if lt % 2 == 0 else "av"),
                         name=f"vp{lt}")
            for j in range(NJ):
                nc.tensor.matmul(ps[:, :DO], xv[j][:, lt * 128:lt * 128 + 128],
                                 wv_sb[:, j, :],
                                 start=(j == 0), stop=(j == NJ - 1))
            nc.vector.tensor_copy(
                v_sb[:, lt, :, 0:DK],
                ps[:, :DO].rearrange("p (h d) -> p h d", h=HG))

        def av_sub(gi, sub):
            p, br, hh, c = GROUPS[gi]
            h = 2 * p + hh
            tag = ("pp" if (gi >= 28 and sub % 2 == 1) else "av")
            av = wk.tile([128, DK + 1], F32, tag=tag, name=f"av{gi}_{sub}")
            for lkt in range(NLT):
                e_t, slot = emap[gi * NLT + lkt]
                nc.tensor.matmul(
                    av[:],
                    e_t[:, slot * 512 + sub * 128:slot * 512 + sub * 128 + 128],
                    v_sb[:, lkt, h % HG, :],
                    start=(lkt == 0), stop=(lkt == NLT - 1))
            # one fast copy frees the PSUM bank; normalize from the copy so
            # the next av accumulation never waits on the norm round-trip
            avc = norm.tile([128, DK + 1], F32, tag="avc", name=f"avc{gi}_{sub}")
            nc.vector.tensor_copy(avc[:], av[:])
            rd = norm.tile([128, 1], F32, tag="rd", name=f"rd{gi}_{sub}")
            nc.vector.reciprocal(rd[:], avc[:, DK:DK + 1])
            if br == 0:
                nc.vector.tensor_scalar_mul(
                    x0_sb[:, p, hh, c, sub, :], avc[:, 0:DK], rd[:])
            else:
                nc.vector.scalar_tensor_tensor(
                    x1_sb[:, hh, c, sub, :], avc[:, 0:DK], rd[:],
                    x0_sb[:, p, hh, c, sub, :],
                    mybir.AluOpType.mult, mybir.AluOpType.add)

        def transp(p, c, sub):
            lt = c * 4 + sub
            tp = wk.tile([128, 128], BF16, tag="av", name=f"tp{p}_{lt}")
            for hh in range(2):
                nc.tensor.matmul(tp[hh * 64:hh * 64 + 64, :],
                                 x1_sb[:, hh, c, sub, :], id_sb[:],
                                 is_transpose=True)
            nc.vector.tensor_copy(xT_sb[:, p, lt, :], tp[:])

        def outproj(lt):
            po = wk.tile([128, 512], F32, tag=("pp" if lt % 2 == 0 else "av"),
                         name=f"po{lt}")
            for p in range(2):
                nc.tensor.matmul(po[:], xT_sb[:, p, lt, :], wo_sb[:, p, :],
                                 start=(p == 0), stop=(p == 1))
            ot = ostage.tile([128, D], F32, tag="ot", name=f"ot{lt}")
            nc.vector.tensor_copy(ot[:], po[:])
            nc.sync.dma_start(out_ap[lt * 128:lt * 128 + 128, :], ot[:])

        # ---- group sequence and chunk stream ----
        GROUPS = []
        for br in range(2):
            for p in range(2):
                for c in range(4):
                    for hh in range(2):
                        GROUPS.append((p, br, hh, c))

        emap = {}
        st = {"stg": None, "et": None, "w": 0}

        # quantum scheduler state
        HIQ = deque()   # (cycles, fn) latency-sensitive: av, transp, outproj
        LOQ = []        # (deadline_chunk, earliest_chunk, cycles, fn), dl-sorted
        credit = [0.0]
        turn = ["lo"]

        def lo_pop_ready(cid):
            for i, (dl, est, cyc, fn) in enumerate(LOQ):
                if est <= cid:
                    LOQ.pop(i)
                    return cyc, fn
                if dl > cid + 40:
                    break
            return None

        def pump(budget, cid):
            credit[0] = min(credit[0] + budget, 8000.0)
            nlo = nhi = 0
            while credit[0] > 0 and (HIQ or LOQ) and nlo + nhi < 4:
                did = False
                order = ("lo", "hi") if turn[0] == "lo" else ("hi", "lo")
                for pref in order:
                    if pref == "lo" and LOQ and nlo < 1:
                        got = lo_pop_ready(cid)
                        if got is None:
                            continue
                        cyc, fn = got
                        turn[0] = "hi"
                        nlo += 1
                    elif pref == "hi" and HIQ and nhi < 2:
                        cyc, fn = HIQ.popleft()
                        turn[0] = "lo"
                        nhi += 1
                    else:
                        continue
                    fn()
                    credit[0] -= cyc
                    did = True
                    break
                if not did:
                    break

        def force_due(cid):
            # at most one forced pop per chunk so score matmuls interleave
            # and cover the single-bank drain latency
            if LOQ and LOQ[0][0] <= cid:
                _, _, cyc, fn = LOQ.pop(0)
                fn()
                credit[0] -= cyc

        def emit_chunk(cid):
            gi, lkt = divmod(cid, NLT)
            p, br, hh, c = GROUPS[gi]
            kT = kgT_sb if br == 0 else klT_sb
            pos = cid % PAIR
            if pos == 0:
                st["stg"] = stg.tile([128, WA], F32, tag="sa", name=f"sa{cid}")
                st["et"] = et.tile([128, WA], BF16, tag="ea", name=f"ea{cid}")
                st["w"] = CA
            elif pos == CA:
                st["stg"] = stg.tile([128, WB], F32, tag="sb", name=f"sb{cid}")
                st["et"] = et.tile([128, WB], BF16, tag="eb", name=f"eb{cid}")
                st["w"] = CB
            slot = pos if pos < CA else pos - CA
            if _DBG:
                print(f"chunk {cid} (gi {gi} lkt {lkt})")
            force_due(cid)
            nc.tensor.matmul(
                st["stg"][:, slot * 512:slot * 512 + 512],
                kT[hh * 64:hh * 64 + 64, p, lkt * 128:lkt * 128 + 128],
                qT_sb[hh * 64:hh * 64 + 64, p, c * 512:c * 512 + 512],
                start=True, stop=True)
            emap[cid] = (st["et"], slot)
            if slot == st["w"] - 1 or cid == NCHUNK - 1:
                w = (slot + 1) * 512
                nc.scalar.activation(st["et"][:, :w], st["stg"][:, :w],
                                     mybir.ActivationFunctionType.Exp)
                act_cyc = (w * 0.8333 + 185.0) / 0.41666
                pump(act_cyc - w, cid)

        # ---- LOW queue: projections, (deadline, earliest) in chunk units ----
        import bisect

        def lo(cyc, fn, dl, est=0):
            if _DBG:
                fn0 = fn

                def fn(fn0=fn0, dl=dl):
                    print(f"  LO pop dl={dl} {fn0}")
                    fn0()
            bisect.insort(LOQ, (dl, est, cyc, fn), key=lambda x: x[0])

        for qb in range(1, 4):
            lo(2048, (lambda qb=qb: proj_chunk(kgT_sb, wkg_sb, 0, qb, kx, off=2,
                                               ptag=("av" if qb % 2 else "pp"))),
               4 * qb, 0)
        for lt in range(16):
            lo(1024, (lambda lt=lt: v_chunk(lt)), 14 + 2 * lt, 12)
        for qb in range(1, 4):
            lo(2048, (lambda qb=qb: proj_chunk(qT_sb, wq_sb, 0, qb, xq,
                                               ptag=("av" if qb == 1 else "pp"))),
               32 * qb - 6, 18 if qb == 1 else 34)
        for i in range(20):
            qb, t = divmod(i, 5)
            if t == 0:
                s = {}
            lo(2048, (lambda s=s, qb=qb, t=t: klT_burst(s, 0, qb, t)),
               30 + 10 * i, 24)
        for qb in range(4):
            lo(2048, (lambda qb=qb: proj_chunk(qT_sb, wq_sb, 1, qb, xq,
                                               ptag=("av" if qb % 2 else "pp"))),
               100 + 6 * qb, 30)
        for qb in range(4):
            lo(2048, (lambda qb=qb: proj_chunk(kgT_sb, wkg_sb, 1, qb, kx, off=2,
                                               ptag=("av" if qb % 2 else "pp"))),
               70 + 6 * qb, 16)
        for i in range(20):
            qb, t = divmod(i, 5)
            if t == 0:
                s2 = {}
            lo(2048, (lambda s2=s2, qb=qb, t=t: klT_burst(s2, 1, qb, t)),
               250 + 5 * i, 150)

        # HIGH pushes happen at group completion: precompute per-group
        def push_av_group(gi):
            for sub in range(4):
                def avfn(gi=gi, sub=sub):
                    if _DBG:
                        print(f"  HI pop av({gi},{sub})")
                    av_sub(gi, sub)
                HIQ.append((1040, avfn))
            p, br, hh, c = GROUPS[gi]
            if br == 1 and hh == 1:
                for sub in range(4):
                    HIQ.append((300, lambda p=p, c=c, sub=sub: transp(p, c, sub)))
                if p == 1:
                    for lt in range(c * 4, c * 4 + 4):
                        HIQ.append((1024, lambda lt=lt: outproj(lt)))

        # av push lag in groups: 3 early (v DMA must land first), then 1,
        # then 0 at the end so the tail chain is minimal
        def av_push_gi(gi):
            return gi + 2 if gi < 8 else (gi + 1 if gi < 28 else gi)

        # ---- prologue projections (rest of kgT m0 arrives via LOW DLs) ----
        proj_chunk(kgT_sb, wkg_sb, 0, 0, kx, off=2)
        proj_chunk(qT_sb, wq_sb, 0, 0, xq)

        # ---- main stream ----
        pushed = set()
        for cid in range(NCHUNK):
            emit_chunk(cid)
            if cid % NLT == NLT - 1:
                gdone = cid // NLT
                for g in range(32):
                    if g not in pushed and av_push_gi(g) <= gdone:
                        push_av_group(g)
                        pushed.add(g)

        # ---- tail ----
        for g in range(32):
            if g not in pushed:
                push_av_group(g)
                pushed.add(g)
        while HIQ or LOQ:
            if LOQ:
                _, _, _, fn = LOQ.pop(0)
            else:
                _, fn = HIQ.popleft()
            fn()

    nc.compile()
    return nc


def _host_prep(inputs):
    """Fold conv+bn+biases; build the 8 per-core input maps."""
    f32 = np.float32
    q = np.ascontiguousarray(inputs["query"], dtype=f32)
    k = np.ascontiguousarray(inputs["key"], dtype=f32)
    v = np.ascontiguousarray(inputs["value"], dtype=f32)
    w3 = np.asarray(inputs["conv_w3"], f32)
    w5 = np.asarray(inputs["conv_w5"], f32)
    b3 = np.asarray(inputs["conv_b3"], f32)
    b5 = np.asarray(inputs["conv_b5"], f32)
    gam = np.asarray(inputs["bn_gamma"], f32)
    bet = np.asarray(inputs["bn_beta"], f32)
    mu = np.asarray(inputs["bn_mean"], f32)
    var = np.asarray(inputs["bn_var"], f32)
    wq = np.asarray(inputs["wq"], f32)
    bq = np.asarray(inputs["bq"], f32)
    wkl = np.asarray(inputs["wkl"], f32)
    bkl = np.asarray(inputs["bkl"], f32)
    wkg = np.asarray(inputs["wkg"], f32)
    bkg = np.asarray(inputs["bkg"], f32)
    wv = np.asarray(inputs["wv"], f32)
    bv = np.asarray(inputs["bv"], f32)
    wo = np.asarray(inputs["wo"], f32)
    bo = np.asarray(inputs["bo"], f32)

    assert not np.any(bq) and not np.any(bkg), "nonzero q/kg bias unsupported"

    s_bn = gam / np.sqrt(var + BN_EPS)
    shift = np.concatenate([b3, b5]) * s_bn + (bet - mu * s_bn)
    wkl_s = wkl * s_bn[None, :]
    A3 = np.einsum("oc,cit->oit", wkl_s[:, :D], w3)
    A5 = np.einsum("oc,cit->oit", wkl_s[:, D:], w5)
    W5c = A5.copy()
    W5c[:, :, 1:4] += A3
    bkl_eff = wkl @ shift + bkl
    wq_eff = wq / math.sqrt(DK)
    bo_eff = bo + wo @ (2.0 * bv)

    bf = BF16_NP
    ident = np.eye(128, dtype=bf)
    in_maps = []
    for c in range(N_CORES):
        b = c // 2
        hg = c % 2
        sel = slice(hg * DO, hg * DO + DO)
        wo_sel = wo.T[sel, :]
        wo2 = np.ascontiguousarray(
            wo_sel.reshape(2, 2, 64, D).transpose(1, 2, 0, 3).reshape(128, 2, D))
        in_maps.append({
            "xq": np.ascontiguousarray(q[b].T).astype(bf),
            "xk": np.ascontiguousarray(k[b].T).astype(bf),
            "xv": np.ascontiguousarray(v[b].T).astype(bf),
            "wq": np.ascontiguousarray(wq_eff.T[:, sel]).astype(bf),
            "wk5": np.ascontiguousarray(W5c.transpose(2, 1, 0)[:, :, sel]).astype(bf),
            "wkg": np.ascontiguousarray(wkg.T[:, sel]).astype(bf),
            "wv": np.ascontiguousarray(wv.T[:, sel]).astype(bf),
            "wo2": wo2.astype(bf),
            "bkl": np.ascontiguousarray(bkl_eff[sel]).astype(f32),
            "ident": ident,
        })
    return in_maps, bo_eff


def kernel(**inputs) -> np.ndarray:
    if "nc" not in _cache:
        _cache["nc"] = _build_program()
    nc = _cache["nc"]
    in_maps, bo_eff = _host_prep(inputs)
    res = bass_utils.run_bass_kernel_spmd(
        nc, in_maps, core_ids=list(range(N_CORES)))
    out = np.zeros((B, L, D), np.float32)
    for c in range(N_CORES):
        out[c // 2] += res.results[c]["out"]
    out += bo_eff[None, None, :]
    return out


# revision 7
# speedup vs baseline: 1.0681x; 1.0168x over previous
"""Trainium2 Bass kernel v2 for nn_MultiHeadedAttention_4269197492266.

Same math as v1 (folded 5-tap conv local-key path, batch x head-group
sharding, ones-column softmax denominator), restructured around the
cost model:

- Score chunks stream continuously into [128,1536] PSUM staging tiles
  (two 3-bank buffers, ping-pong); exp runs as 171 wide ACT instructions (~250us, the
  ACT floor for 33.5M exps/core).  All other PE work (projections, conv
  taps, AV, transposes, outproj) is queued as sub-microsecond quanta and
  popped between staging tiles under a cycle budget with emission
  deadlines, so the in-order PE queue never starves ACT.
- AV is reoriented: out[lq, dk+1] = e_chunk^T @ V -- each matmul streams
  65 columns instead of 512, halving AV cost.  Group-major score order
  makes AV accumulators sequential: one PSUM bank suffices.
- The softmax denominator lands in column 64 per lq-partition:
  normalization is reciprocal + per-partition tensor_scalar on DVE.
- Branches combine before the output projection (scalar_tensor_tensor),
  halving outproj; x returns to [hdk, lq] via PE transposes (host-fed
  identity matrix).
"""

import math
import os
from collections import deque
from contextlib import ExitStack

_DBG = os.environ.get("KV2_DEBUG") == "1"

import ml_dtypes
import numpy as np

import concourse.tile as tile
from concourse import bacc, mybir
from concourse import bass_utils

F32 = mybir.dt.float32
BF16 = mybir.dt.bfloat16
BF16_NP = ml_dtypes.bfloat16

B, L, D = 4, 2048, 512
H, DK = 8, 64
N_CORES = 8
HG = 4
DO = HG * DK
BN_EPS = 1e-5
NJ = D // 128
NLT = L // 128
WA, WB = 1536, 1536          # alternating staging widths
CA, CB = WA // 512, WB // 512  # chunks per tile: 4, 2
PAIR = CA + CB               # 6 chunks per A/B pair
NCHUNK = 512                 # 2p * 2br * 2hh * 4c * 16lkt / ... = 32 groups * 16

_cache = {}


def _build_program():
    nc = bacc.Bacc("TRN2", target_bir_lowering=False, debug=False,
                   num_devices=N_CORES)

    dt_in = {}
    for nm in ("xq", "xk", "xv"):
        dt_in[nm] = nc.dram_tensor(nm, [D, L], BF16, kind="ExternalInput").ap()
    for nm in ("wq", "wkg", "wv"):
        dt_in[nm] = nc.dram_tensor(nm, [D, DO], BF16, kind="ExternalInput").ap()
    dt_in["wk5"] = nc.dram_tensor("wk5", [5, D, DO], BF16, kind="ExternalInput").ap()
    dt_in["wo2"] = nc.dram_tensor("wo2", [128, 2, D], BF16, kind="ExternalInput").ap()
    dt_in["bkl"] = nc.dram_tensor("bkl", [DO], F32, kind="ExternalInput").ap()
    dt_in["ident"] = nc.dram_tensor("ident", [128, 128], BF16, kind="ExternalInput").ap()
    out_ap = nc.dram_tensor("out", [L, D], F32, kind="ExternalOutput").ap()

    with tile.TileContext(nc) as tc, ExitStack() as ctx:
        big = ctx.enter_context(tc.tile_pool(name="big", bufs=12))
        et = ctx.enter_context(tc.tile_pool(name="et", bufs=12))
        proj = ctx.enter_context(tc.tile_pool(name="projsb", bufs=1))
        norm = ctx.enter_context(tc.tile_pool(name="norm", bufs=8))
        ostage = ctx.enter_context(tc.tile_pool(name="ostage", bufs=3))
        stg = ctx.enter_context(tc.tile_pool(name="stg", bufs=1, space="PSUM"))
        wk = ctx.enter_context(tc.tile_pool(name="wk", bufs=1, space="PSUM"))

        # ---- persistent SBUF tensors ----
        wq_sb = proj.tile([128, NJ, DO], BF16, tag="wq")
        wk5_sb = proj.tile([128, 5, NJ, DO], BF16, tag="wk5")
        wkg_sb = proj.tile([128, NJ, DO], BF16, tag="wkg")
        wv_sb = proj.tile([128, NJ, DO], BF16, tag="wv")
        wo_sb = proj.tile([128, 2, D], BF16, tag="wo")
        id_sb = proj.tile([128, 128], BF16, tag="ident")
        bkl_sb = proj.tile([128, 2], F32, tag="bkl")
        qT_sb = proj.tile([128, 2, L], BF16, tag="qT")
        klT_sb = proj.tile([128, 2, L], BF16, tag="klT")
        kgT_sb = proj.tile([128, 2, L], BF16, tag="kgT")
        v_sb = proj.tile([128, NLT, HG, DK + 1], BF16, tag="v")
        x0_sb = proj.tile([128, 2, 2, 4, 4, DK], BF16, tag="x0")
        x1_sb = proj.tile([128, 2, 4, 4, DK], BF16, tag="x1")
        xT_sb = proj.tile([128, 2, NLT, 128], BF16, tag="xT")
        zw_sb = proj.tile([128, 512], BF16, tag="zw")

        # ---- input DMA: few big ops, 3 queues, need-by ordering ----
        LKP = L + 4
        kxall = big.tile([128, NJ, LKP], BF16, tag="kx", name="kxall", bufs=1)
        xqall = big.tile([128, NJ, L], BF16, tag="xq", name="xqall", bufs=1)
        xvall = big.tile([128, NJ, L], BF16, tag="xv", name="xvall", bufs=1)
        kx = [kxall[:, j, :] for j in range(NJ)]
        xq = [xqall[:, j, :] for j in range(NJ)]
        xv = [xvall[:, j, :] for j in range(NJ)]
        nc.vector.memset(kxall[:, :, 0:2], 0.0)
        nc.vector.memset(kxall[:, :, 2 + L:], 0.0)
        xk_r = dt_in["xk"].rearrange("(j p) l -> p j l", p=128)
        xq_r = dt_in["xq"].rearrange("(j p) l -> p j l", p=128)
        xv_r = dt_in["xv"].rearrange("(j p) l -> p j l", p=128)
        # The cost model serializes all DMA transfers on one resource, so
        # queue parallelism buys nothing: issue everything on one queue in
        # exact need order, sized so each lands just before its consumer.
        nc.sync.dma_start(kxall[:, :, 2:518], xk_r[:, :, 0:516])        # kgT qb0
        nc.sync.dma_start(wkg_sb[:], dt_in["wkg"].rearrange("(j p) o -> p j o", p=128))
        nc.sync.dma_start(wq_sb[:], dt_in["wq"].rearrange("(j p) o -> p j o", p=128))
        nc.sync.dma_start(xqall[:, :, 0:512], xq_r[:, :, 0:512])        # qT c0
        nc.sync.dma_start(kxall[:, :, 518:1034], xk_r[:, :, 516:1032])  # kgT qb1
        nc.sync.dma_start(kxall[:, :, 1034:2 + L], xk_r[:, :, 1032:])   # kgT qb2-3
        nc.sync.dma_start(xvall[:, 0:2, :], xv_r[:, 0:2, :])
        nc.sync.dma_start(wv_sb[:], dt_in["wv"].rearrange("(j p) o -> p j o", p=128))
        nc.sync.dma_start(xvall[:, 2:4, :], xv_r[:, 2:4, :])
        nc.sync.dma_start(xqall[:, :, 512:1024], xq_r[:, :, 512:1024])  # qT c1
        nc.sync.dma_start(wk5_sb[:], dt_in["wk5"].rearrange("t (j p) o -> p t j o", p=128))
        nc.sync.dma_start(xqall[:, :, 1024:], xq_r[:, :, 1024:])        # qT c2-3
        nc.sync.dma_start(bkl_sb[:], dt_in["bkl"].rearrange("(m p) -> p m", p=128))
        nc.sync.dma_start(wo_sb[:], dt_in["wo2"])
        nc.sync.dma_start(id_sb[:], dt_in["ident"])

        warm = proj.tile([1, 16], F32, tag="warmt")
        nc.vector.memset(warm[:], 0.0)
        nc.scalar.activation(warm[:], warm[:], mybir.ActivationFunctionType.Exp)

        # ---- PE warm-up: ramp the p-state while input DMA lands ----
        nc.vector.memset(zw_sb[:], 0.0)
        for i in range(12):
            zp = wk.tile([128, 512], F32, tag="pp", name=f"zp{i}")
            nc.tensor.matmul(zp[:], zw_sb[:, 0:128], zw_sb[:], start=True, stop=True)

        # ---- emitters ----
        def proj_chunk(dst_sb, w_sb, m, qb, src, bias=None, off=0, ptag="pp"):
            ps = wk.tile([128, 512], F32, tag=ptag, name=f"pp{m}_{qb}")
            for j in range(NJ):
                nc.tensor.matmul(ps[:], w_sb[:, j, m * 128:(m + 1) * 128],
                                 src[j][:, off + qb * 512:off + qb * 512 + 512],
                                 start=(j == 0), stop=(j == NJ - 1))
            if bias is not None:
                nc.vector.tensor_scalar_add(
                    dst_sb[:, m, qb * 512:qb * 512 + 512], ps[:], bias[:, m:m + 1])
            else:
                nc.vector.tensor_copy(dst_sb[:, m, qb * 512:qb * 512 + 512], ps[:])

        def klT_burst(state, m, qb, t):
            if t == 0:
                tg = "av" if (m == 0 and qb == 0) else "pp"
                state["ps"] = wk.tile([128, 512], F32, tag=tg, name=f"kl{m}_{qb}")
            ps = state["ps"]
            for j in range(NJ):
                nc.tensor.matmul(ps[:], wk5_sb[:, t, j, m * 128:(m + 1) * 128],
                                 kx[j][:, qb * 512 + t:qb * 512 + t + 512],
                                 start=(t == 0 and j == 0), stop=(t == 4 and j == NJ - 1))
            if t == 4:
                nc.vector.tensor_scalar_add(
                    klT_sb[:, m, qb * 512:qb * 512 + 512], ps[:], bkl_sb[:, m:m + 1])

        def v_chunk(lt):
            if lt == 0:
                nc.vector.memset(v_sb[:], 1.0)
            ps = wk.tile([128, 512], F32, tag=("pp" if lt % 2 == 0 else "av"),
                         name=f"vp{lt}")
            for j in range(NJ):
                nc.tensor.matmul(ps[:, :DO], xv[j][:, lt * 128:lt * 128 + 128],
                                 wv_sb[:, j, :],
                                 start=(j == 0), stop=(j == NJ - 1))
            nc.vector.tensor_copy(
                v_sb[:, lt, :, 0:DK],
                ps[:, :DO].rearrange("p (h d) -> p h d", h=HG))

        def av_sub(gi, sub):
            p, br, hh, c = GROUPS[gi]
            h = 2 * p + hh
            tag = ("pp" if (gi >= 28 and sub % 2 == 1) else "av")
            av = wk.tile([128, DK + 1], F32, tag=tag, name=f"av{gi}_{sub}")
            for lkt in range(NLT):
                e_t, slot = emap[gi * NLT + lkt]
                nc.tensor.matmul(
                    av[:],
                    e_t[:, slot * 512 + sub * 128:slot * 512 + sub * 128 + 128],
                    v_sb[:, lkt, h % HG, :],
                    start=(lkt == 0), stop=(lkt == NLT - 1))
            # one fast copy frees the PSUM bank; normalize from the copy so
            # the next av accumulation never waits on the norm round-trip
            avc = norm.tile([128, DK + 1], F32, tag="avc", name=f"avc{gi}_{sub}")
            nc.vector.tensor_copy(avc[:], av[:])
            rd = norm.tile([128, 1], F32, tag="rd", name=f"rd{gi}_{sub}")
            nc.vector.reciprocal(rd[:], avc[:, DK:DK + 1])
            if br == 0:
                nc.vector.tensor_scalar_mul(
                    x0_sb[:, p, hh, c, sub, :], avc[:, 0:DK], rd[:])
            else:
                nc.vector.scalar_tensor_tensor(
                    x1_sb[:, hh, c, sub, :], avc[:, 0:DK], rd[:],
                    x0_sb[:, p, hh, c, sub, :],
                    mybir.AluOpType.mult, mybir.AluOpType.add)

        def transp(p, c, sub):
            lt = c * 4 + sub
            tp = wk.tile([128, 128], BF16, tag="av", name=f"tp{p}_{lt}")
            for hh in range(2):
                nc.tensor.matmul(tp[hh * 64:hh * 64 + 64, :],
                                 x1_sb[:, hh, c, sub, :], id_sb[:],
                                 is_transpose=True)
            nc.vector.tensor_copy(xT_sb[:, p, lt, :], tp[:])

        def outproj(lt):
            po = wk.tile([128, 512], F32, tag=("pp" if lt % 2 == 0 else "av"),
                         name=f"po{lt}")
            for p in range(2):
                nc.tensor.matmul(po[:], xT_sb[:, p, lt, :], wo_sb[:, p, :],
                                 start=(p == 0), stop=(p == 1))
            ot = ostage.tile([128, D], F32, tag="ot", name=f"ot{lt}")
            nc.vector.tensor_copy(ot[:], po[:])
            nc.sync.dma_start(out_ap[lt * 128:lt * 128 + 128, :], ot[:])

        # ---- group sequence and chunk stream ----
        GROUPS = []
        for br in range(2):
            for p in range(2):
                for c in range(4):
                    for hh in range(2):
                        GROUPS.append((p, br, hh, c))

        emap = {}
        st = {"stg": None, "et": None, "w": 0}

        # quantum scheduler state
        HIQ = deque()   # (cycles, fn) latency-sensitive: av, transp, outproj
        LOQ = []        # (deadline_chunk, earliest_chunk, cycles, fn), dl-sorted
        credit = [0.0]
        turn = ["lo"]

        def lo_pop_ready(cid):
            for i, (dl, est, cyc, fn) in enumerate(LOQ):
                if est <= cid:
                    LOQ.pop(i)
                    return cyc, fn
                if dl > cid + 40:
                    break
            return None

        def pump(budget, cid):
            credit[0] = min(credit[0] + budget, 8000.0)
            nlo = nhi = 0
            while credit[0] > 0 and (HIQ or LOQ) and nlo + nhi < 4:
                did = False
                order = ("lo", "hi") if turn[0] == "lo" else ("hi", "lo")
                for pref in order:
                    if pref == "lo" and LOQ and nlo < 1:
                        got = lo_pop_ready(cid)
                        if got is None:
                            continue
                        cyc, fn = got
                        turn[0] = "hi"
                        nlo += 1
                    elif pref == "hi" and HIQ and nhi < 2:
                        cyc, fn = HIQ.popleft()
                        turn[0] = "lo"
                        nhi += 1
                    else:
                        continue
                    fn()
                    credit[0] -= cyc
                    did = True
                    break
                if not did:
                    break

        def force_due(cid):
            # at most one forced pop per chunk so score matmuls interleave
            # and cover the single-bank drain latency
            if LOQ and LOQ[0][0] <= cid:
                _, _, cyc, fn = LOQ.pop(0)
                fn()
                credit[0] -= cyc

        def emit_chunk(cid):
            gi, lkt = divmod(cid, NLT)
            p, br, hh, c = GROUPS[gi]
            kT = kgT_sb if br == 0 else klT_sb
            pos = cid % PAIR
            if pos == 0:
                st["stg"] = stg.tile([128, WA], F32, tag="sa", name=f"sa{cid}")
                st["et"] = et.tile([128, WA], BF16, tag="ea", name=f"ea{cid}")
                st["w"] = CA
            elif pos == CA:
                st["stg"] = stg.tile([128, WB], F32, tag="sb", name=f"sb{cid}")
                st["et"] = et.tile([128, WB], BF16, tag="eb", name=f"eb{cid}")
                st["w"] = CB
            slot = pos if pos < CA else pos - CA
            if _DBG:
                print(f"chunk {cid} (gi {gi} lkt {lkt})")
            force_due(cid)
            nc.tensor.matmul(
                st["stg"][:, slot * 512:slot * 512 + 512],
                kT[hh * 64:hh * 64 + 64, p, lkt * 128:lkt * 128 + 128],
                qT_sb[hh * 64:hh * 64 + 64, p, c * 512:c * 512 + 512],
                start=True, stop=True)
            emap[cid] = (st["et"], slot)
            if slot == st["w"] - 1 or cid == NCHUNK - 1:
                w = (slot + 1) * 512
                nc.scalar.activation(st["et"][:, :w], st["stg"][:, :w],
                                     mybir.ActivationFunctionType.Exp)
                act_cyc = (w * 0.8333 + 185.0) / 0.41666
                pump(act_cyc - w, cid)

        # ---- LOW queue: projections, (deadline, earliest) in chunk units ----
        import bisect

        def lo(cyc, fn, dl, est=0):
            if _DBG:
                fn0 = fn

                def fn(fn0=fn0, dl=dl):
                    print(f"  LO pop dl={dl} {fn0}")
                    fn0()
            bisect.insort(LOQ, (dl, est, cyc, fn), key=lambda x: x[0])

        for qb in range(1, 4):
            lo(2048, (lambda qb=qb: proj_chunk(kgT_sb, wkg_sb, 0, qb, kx, off=2,
                                               ptag=("av" if qb % 2 else "pp"))),
               4 * qb, 0)
        for lt in range(16):
            lo(1024, (lambda lt=lt: v_chunk(lt)), 14 + 2 * lt, 12)
        for qb in range(1, 4):
            lo(2048, (lambda qb=qb: proj_chunk(qT_sb, wq_sb, 0, qb, xq,
                                               ptag=("av" if qb == 1 else "pp"))),
               32 * qb - 6, 18 if qb == 1 else 34)
        for i in range(20):
            qb, t = divmod(i, 5)
            if t == 0:
                s = {}
            lo(2048, (lambda s=s, qb=qb, t=t: klT_burst(s, 0, qb, t)),
               30 + 10 * i, 24)
        for qb in range(4):
            lo(2048, (lambda qb=qb: proj_chunk(qT_sb, wq_sb, 1, qb, xq,
                                               ptag=("av" if qb % 2 else "pp"))),
               120 + 32 * qb, 104 + 32 * qb)
        for qb in range(4):
            lo(2048, (lambda qb=qb: proj_chunk(kgT_sb, wkg_sb, 1, qb, kx, off=2,
                                               ptag=("av" if qb % 2 else "pp"))),
               70 + 6 * qb, 56 + 6 * qb)
        for i in range(20):
            qb, t = divmod(i, 5)
            if t == 0:
                s2 = {}
            lo(2048, (lambda s2=s2, qb=qb, t=t: klT_burst(s2, 1, qb, t)),
               250 + 5 * i, 150)

        # HIGH pushes happen at group completion: precompute per-group
        def push_av_group(gi):
            for sub in range(4):
                def avfn(gi=gi, sub=sub):
                    if _DBG:
                        print(f"  HI pop av({gi},{sub})")
                    av_sub(gi, sub)
                HIQ.append((1040, avfn))
            p, br, hh, c = GROUPS[gi]
            if br == 1 and hh == 1:
                for sub in range(4):
                    HIQ.append((300, lambda p=p, c=c, sub=sub: transp(p, c, sub)))
                if p == 1:
                    for lt in range(c * 4, c * 4 + 4):
                        HIQ.append((1024, lambda lt=lt: outproj(lt)))

        # av push lag in groups: 3 early (v DMA must land first), then 1,
        # then 0 at the end so the tail chain is minimal
        def av_push_gi(gi):
            return gi + 2 if gi < 8 else (gi + 1 if gi < 28 else gi)

        # ---- prologue projections (rest of kgT m0 arrives via LOW DLs) ----
        proj_chunk(kgT_sb, wkg_sb, 0, 0, kx, off=2)
        proj_chunk(qT_sb, wq_sb, 0, 0, xq)

        # ---- main stream ----
        pushed = set()
        for cid in range(NCHUNK):
            emit_chunk(cid)
            if cid % NLT == NLT - 1:
                gdone = cid // NLT
                for g in range(32):
                    if g not in pushed and av_push_gi(g) <= gdone:
                        push_av_group(g)
                        pushed.add(g)

        # ---- tail ----
        for g in range(32):
            if g not in pushed:
                push_av_group(g)
                pushed.add(g)
        while HIQ or LOQ:
            if LOQ:
                _, _, _, fn = LOQ.pop(0)
            else:
                _, fn = HIQ.popleft()
            fn()

    nc.compile()
    return nc


def _host_prep(inputs):
    """Fold conv+bn+biases; build the 8 per-core input maps."""
    f32 = np.float32
    q = np.ascontiguousarray(inputs["query"], dtype=f32)
    k = np.ascontiguousarray(inputs["key"], dtype=f32)
    v = np.ascontiguousarray(inputs["value"], dtype=f32)
    w3 = np.asarray(inputs["conv_w3"], f32)
    w5 = np.asarray(inputs["conv_w5"], f32)
    b3 = np.asarray(inputs["conv_b3"], f32)
    b5 = np.asarray(inputs["conv_b5"], f32)
    gam = np.asarray(inputs["bn_gamma"], f32)
    bet = np.asarray(inputs["bn_beta"], f32)
    mu = np.asarray(inputs["bn_mean"], f32)
    var = np.asarray(inputs["bn_var"], f32)
    wq = np.asarray(inputs["wq"], f32)
    bq = np.asarray(inputs["bq"], f32)
    wkl = np.asarray(inputs["wkl"], f32)
    bkl = np.asarray(inputs["bkl"], f32)
    wkg = np.asarray(inputs["wkg"], f32)
    bkg = np.asarray(inputs["bkg"], f32)
    wv = np.asarray(inputs["wv"], f32)
    bv = np.asarray(inputs["bv"], f32)
    wo = np.asarray(inputs["wo"], f32)
    bo = np.asarray(inputs["bo"], f32)

    assert not np.any(bq) and not np.any(bkg), "nonzero q/kg bias unsupported"

    s_bn = gam / np.sqrt(var + BN_EPS)
    shift = np.concatenate([b3, b5]) * s_bn + (bet - mu * s_bn)
    wkl_s = wkl * s_bn[None, :]
    A3 = np.einsum("oc,cit->oit", wkl_s[:, :D], w3)
    A5 = np.einsum("oc,cit->oit", wkl_s[:, D:], w5)
    W5c = A5.copy()
    W5c[:, :, 1:4] += A3
    bkl_eff = wkl @ shift + bkl
    wq_eff = wq / math.sqrt(DK)
    bo_eff = bo + wo @ (2.0 * bv)

    bf = BF16_NP
    ident = np.eye(128, dtype=bf)
    in_maps = []
    for c in range(N_CORES):
        b = c // 2
        hg = c % 2
        sel = slice(hg * DO, hg * DO + DO)
        wo_sel = wo.T[sel, :]
        wo2 = np.ascontiguousarray(
            wo_sel.reshape(2, 2, 64, D).transpose(1, 2, 0, 3).reshape(128, 2, D))
        in_maps.append({
            "xq": np.ascontiguousarray(q[b].T).astype(bf),
            "xk": np.ascontiguousarray(k[b].T).astype(bf),
            "xv": np.ascontiguousarray(v[b].T).astype(bf),
            "wq": np.ascontiguousarray(wq_eff.T[:, sel]).astype(bf),
            "wk5": np.ascontiguousarray(W5c.transpose(2, 1, 0)[:, :, sel]).astype(bf),
            "wkg": np.ascontiguousarray(wkg.T[:, sel]).astype(bf),
            "wv": np.ascontiguousarray(wv.T[:, sel]).astype(bf),
            "wo2": wo2.astype(bf),
            "bkl": np.ascontiguousarray(bkl_eff[sel]).astype(f32),
            "ident": ident,
        })
    return in_maps, bo_eff


def kernel(**inputs) -> np.ndarray:
    if "nc" not in _cache:
        _cache["nc"] = _build_program()
    nc = _cache["nc"]
    in_maps, bo_eff = _host_prep(inputs)
    res = bass_utils.run_bass_kernel_spmd(
        nc, in_maps, core_ids=list(range(N_CORES)))
    out = np.zeros((B, L, D), np.float32)
    for c in range(N_CORES):
        out[c // 2] += res.results[c]["out"]
    out += bo_eff[None, None, :]
    return out


# revision 8
# speedup vs baseline: 1.0705x; 1.0023x over previous
"""Trainium2 Bass kernel v2 for nn_MultiHeadedAttention_4269197492266.

Same math as v1 (folded 5-tap conv local-key path, batch x head-group
sharding, ones-column softmax denominator), restructured around the
cost model:

- Score chunks stream continuously into [128,1536] PSUM staging tiles
  (two 3-bank buffers, ping-pong); exp runs as 171 wide ACT instructions (~250us, the
  ACT floor for 33.5M exps/core).  All other PE work (projections, conv
  taps, AV, transposes, outproj) is queued as sub-microsecond quanta and
  popped between staging tiles under a cycle budget with emission
  deadlines, so the in-order PE queue never starves ACT.
- AV is reoriented: out[lq, dk+1] = e_chunk^T @ V -- each matmul streams
  65 columns instead of 512, halving AV cost.  Group-major score order
  makes AV accumulators sequential: one PSUM bank suffices.
- The softmax denominator lands in column 64 per lq-partition:
  normalization is reciprocal + per-partition tensor_scalar on DVE.
- Branches combine before the output projection (scalar_tensor_tensor),
  halving outproj; x returns to [hdk, lq] via PE transposes (host-fed
  identity matrix).
"""

import math
import os
from collections import deque
from contextlib import ExitStack

_DBG = os.environ.get("KV2_DEBUG") == "1"

import ml_dtypes
import numpy as np

import concourse.tile as tile
from concourse import bacc, mybir
from concourse import bass_utils

F32 = mybir.dt.float32
BF16 = mybir.dt.bfloat16
BF16_NP = ml_dtypes.bfloat16

B, L, D = 4, 2048, 512
H, DK = 8, 64
N_CORES = 8
HG = 4
DO = HG * DK
BN_EPS = 1e-5
NJ = D // 128
NLT = L // 128
WA, WB = 1536, 1536          # alternating staging widths
CA, CB = WA // 512, WB // 512  # chunks per tile: 4, 2
PAIR = CA + CB               # 6 chunks per A/B pair
NCHUNK = 512                 # 2p * 2br * 2hh * 4c * 16lkt / ... = 32 groups * 16

_cache = {}


def _build_program():
    nc = bacc.Bacc("TRN2", target_bir_lowering=False, debug=False,
                   num_devices=N_CORES)

    dt_in = {}
    for nm in ("xq", "xk", "xv"):
        dt_in[nm] = nc.dram_tensor(nm, [D, L], BF16, kind="ExternalInput").ap()
    for nm in ("wq", "wkg", "wv"):
        dt_in[nm] = nc.dram_tensor(nm, [D, DO], BF16, kind="ExternalInput").ap()
    dt_in["wk5"] = nc.dram_tensor("wk5", [5, D, DO], BF16, kind="ExternalInput").ap()
    dt_in["wo2"] = nc.dram_tensor("wo2", [128, 2, D], BF16, kind="ExternalInput").ap()
    dt_in["bkl"] = nc.dram_tensor("bkl", [DO], F32, kind="ExternalInput").ap()
    dt_in["ident"] = nc.dram_tensor("ident", [128, 128], BF16, kind="ExternalInput").ap()
    out_ap = nc.dram_tensor("out", [L, D], BF16, kind="ExternalOutput").ap()

    with tile.TileContext(nc) as tc, ExitStack() as ctx:
        big = ctx.enter_context(tc.tile_pool(name="big", bufs=12))
        et = ctx.enter_context(tc.tile_pool(name="et", bufs=12))
        proj = ctx.enter_context(tc.tile_pool(name="projsb", bufs=1))
        norm = ctx.enter_context(tc.tile_pool(name="norm", bufs=8))
        ostage = ctx.enter_context(tc.tile_pool(name="ostage", bufs=3))
        stg = ctx.enter_context(tc.tile_pool(name="stg", bufs=1, space="PSUM"))
        wk = ctx.enter_context(tc.tile_pool(name="wk", bufs=1, space="PSUM"))

        # ---- persistent SBUF tensors ----
        wq_sb = proj.tile([128, NJ, DO], BF16, tag="wq")
        wk5_sb = proj.tile([128, 5, NJ, DO], BF16, tag="wk5")
        wkg_sb = proj.tile([128, NJ, DO], BF16, tag="wkg")
        wv_sb = proj.tile([128, NJ, DO], BF16, tag="wv")
        wo_sb = proj.tile([128, 2, D], BF16, tag="wo")
        id_sb = proj.tile([128, 128], BF16, tag="ident")
        bkl_sb = proj.tile([128, 2], F32, tag="bkl")
        qT_sb = proj.tile([128, 2, L], BF16, tag="qT")
        klT_sb = proj.tile([128, 2, L], BF16, tag="klT")
        kgT_sb = proj.tile([128, 2, L], BF16, tag="kgT")
        v_sb = proj.tile([128, NLT, HG, DK + 1], BF16, tag="v")
        x0_sb = proj.tile([128, 2, 2, 4, 4, DK], BF16, tag="x0")
        x1_sb = proj.tile([128, 2, 4, 4, DK], BF16, tag="x1")
        xT_sb = proj.tile([128, 2, NLT, 128], BF16, tag="xT")
        zw_sb = proj.tile([128, 512], BF16, tag="zw")

        # ---- input DMA: few big ops, 3 queues, need-by ordering ----
        LKP = L + 4
        kxall = big.tile([128, NJ, LKP], BF16, tag="kx", name="kxall", bufs=1)
        xqall = big.tile([128, NJ, L], BF16, tag="xq", name="xqall", bufs=1)
        xvall = big.tile([128, NJ, L], BF16, tag="xv", name="xvall", bufs=1)
        kx = [kxall[:, j, :] for j in range(NJ)]
        xq = [xqall[:, j, :] for j in range(NJ)]
        xv = [xvall[:, j, :] for j in range(NJ)]
        nc.vector.memset(kxall[:, :, 0:2], 0.0)
        nc.vector.memset(kxall[:, :, 2 + L:], 0.0)
        xk_r = dt_in["xk"].rearrange("(j p) l -> p j l", p=128)
        xq_r = dt_in["xq"].rearrange("(j p) l -> p j l", p=128)
        xv_r = dt_in["xv"].rearrange("(j p) l -> p j l", p=128)
        # The cost model serializes all DMA transfers on one resource, so
        # queue parallelism buys nothing: issue everything on one queue in
        # exact need order, sized so each lands just before its consumer.
        nc.sync.dma_start(kxall[:, :, 2:518], xk_r[:, :, 0:516])        # kgT qb0
        nc.sync.dma_start(wkg_sb[:], dt_in["wkg"].rearrange("(j p) o -> p j o", p=128))
        nc.sync.dma_start(wq_sb[:], dt_in["wq"].rearrange("(j p) o -> p j o", p=128))
        nc.sync.dma_start(xqall[:, :, 0:512], xq_r[:, :, 0:512])        # qT c0
        nc.sync.dma_start(kxall[:, :, 518:1034], xk_r[:, :, 516:1032])  # kgT qb1
        nc.sync.dma_start(kxall[:, :, 1034:2 + L], xk_r[:, :, 1032:])   # kgT qb2-3
        nc.sync.dma_start(xvall[:, 0:2, :], xv_r[:, 0:2, :])
        nc.sync.dma_start(wv_sb[:], dt_in["wv"].rearrange("(j p) o -> p j o", p=128))
        nc.sync.dma_start(xvall[:, 2:4, :], xv_r[:, 2:4, :])
        nc.sync.dma_start(xqall[:, :, 512:1024], xq_r[:, :, 512:1024])  # qT c1
        nc.sync.dma_start(wk5_sb[:], dt_in["wk5"].rearrange("t (j p) o -> p t j o", p=128))
        nc.sync.dma_start(xqall[:, :, 1024:], xq_r[:, :, 1024:])        # qT c2-3
        nc.sync.dma_start(bkl_sb[:], dt_in["bkl"].rearrange("(m p) -> p m", p=128))
        nc.sync.dma_start(wo_sb[:], dt_in["wo2"])
        nc.sync.dma_start(id_sb[:], dt_in["ident"])

        warm = proj.tile([1, 16], F32, tag="warmt")
        nc.vector.memset(warm[:], 0.0)
        nc.scalar.activation(warm[:], warm[:], mybir.ActivationFunctionType.Exp)

        # ---- PE warm-up: ramp the p-state while input DMA lands ----
        nc.vector.memset(zw_sb[:], 0.0)
        for i in range(12):
            zp = wk.tile([128, 512], F32, tag="pp", name=f"zp{i}")
            nc.tensor.matmul(zp[:], zw_sb[:, 0:128], zw_sb[:], start=True, stop=True)

        # ---- emitters ----
        def proj_chunk(dst_sb, w_sb, m, qb, src, bias=None, off=0, ptag="pp"):
            ps = wk.tile([128, 512], F32, tag=ptag, name=f"pp{m}_{qb}")
            for j in range(NJ):
                nc.tensor.matmul(ps[:], w_sb[:, j, m * 128:(m + 1) * 128],
                                 src[j][:, off + qb * 512:off + qb * 512 + 512],
                                 start=(j == 0), stop=(j == NJ - 1))
            if bias is not None:
                nc.vector.tensor_scalar_add(
                    dst_sb[:, m, qb * 512:qb * 512 + 512], ps[:], bias[:, m:m + 1])
            else:
                nc.vector.tensor_copy(dst_sb[:, m, qb * 512:qb * 512 + 512], ps[:])

        def klT_burst(state, m, qb, t):
            if t == 0:
                tg = "av" if (m == 0 and qb == 0) else "pp"
                state["ps"] = wk.tile([128, 512], F32, tag=tg, name=f"kl{m}_{qb}")
            ps = state["ps"]
            for j in range(NJ):
                nc.tensor.matmul(ps[:], wk5_sb[:, t, j, m * 128:(m + 1) * 128],
                                 kx[j][:, qb * 512 + t:qb * 512 + t + 512],
                                 start=(t == 0 and j == 0), stop=(t == 4 and j == NJ - 1))
            if t == 4:
                nc.vector.tensor_scalar_add(
                    klT_sb[:, m, qb * 512:qb * 512 + 512], ps[:], bkl_sb[:, m:m + 1])

        def v_chunk(lt):
            if lt == 0:
                nc.vector.memset(v_sb[:], 1.0)
            ps = wk.tile([128, 512], F32, tag=("pp" if lt % 2 == 0 else "av"),
                         name=f"vp{lt}")
            for j in range(NJ):
                nc.tensor.matmul(ps[:, :DO], xv[j][:, lt * 128:lt * 128 + 128],
                                 wv_sb[:, j, :],
                                 start=(j == 0), stop=(j == NJ - 1))
            nc.vector.tensor_copy(
                v_sb[:, lt, :, 0:DK],
                ps[:, :DO].rearrange("p (h d) -> p h d", h=HG))

        def av_sub(gi, sub):
            p, br, hh, c = GROUPS[gi]
            h = 2 * p + hh
            tag = ("pp" if (gi >= 28 and sub % 2 == 1) else "av")
            av = wk.tile([128, DK + 1], F32, tag=tag, name=f"av{gi}_{sub}")
            for lkt in range(NLT):
                e_t, slot = emap[gi * NLT + lkt]
                nc.tensor.matmul(
                    av[:],
                    e_t[:, slot * 512 + sub * 128:slot * 512 + sub * 128 + 128],
                    v_sb[:, lkt, h % HG, :],
                    start=(lkt == 0), stop=(lkt == NLT - 1))
            # one fast copy frees the PSUM bank; normalize from the copy so
            # the next av accumulation never waits on the norm round-trip
            avc = norm.tile([128, DK + 1], F32, tag="avc", name=f"avc{gi}_{sub}")
            nc.vector.tensor_copy(avc[:], av[:])
            rd = norm.tile([128, 1], F32, tag="rd", name=f"rd{gi}_{sub}")
            nc.vector.reciprocal(rd[:], avc[:, DK:DK + 1])
            if br == 0:
                nc.vector.tensor_scalar_mul(
                    x0_sb[:, p, hh, c, sub, :], avc[:, 0:DK], rd[:])
            else:
                nc.vector.scalar_tensor_tensor(
                    x1_sb[:, hh, c, sub, :], avc[:, 0:DK], rd[:],
                    x0_sb[:, p, hh, c, sub, :],
                    mybir.AluOpType.mult, mybir.AluOpType.add)

        def transp(p, c, sub):
            lt = c * 4 + sub
            tp = wk.tile([128, 128], BF16, tag="av", name=f"tp{p}_{lt}")
            for hh in range(2):
                nc.tensor.matmul(tp[hh * 64:hh * 64 + 64, :],
                                 x1_sb[:, hh, c, sub, :], id_sb[:],
                                 is_transpose=True)
            nc.vector.tensor_copy(xT_sb[:, p, lt, :], tp[:])

        def outproj(lt):
            po = wk.tile([128, 512], F32, tag=("pp" if lt % 2 == 0 else "av"),
                         name=f"po{lt}")
            for p in range(2):
                nc.tensor.matmul(po[:], xT_sb[:, p, lt, :], wo_sb[:, p, :],
                                 start=(p == 0), stop=(p == 1))
            ot = ostage.tile([128, D], BF16, tag="ot", name=f"ot{lt}")
            nc.vector.tensor_copy(ot[:], po[:])
            nc.sync.dma_start(out_ap[lt * 128:lt * 128 + 128, :], ot[:])

        # ---- group sequence and chunk stream ----
        GROUPS = []
        for br in range(2):
            for p in range(2):
                for c in range(4):
                    for hh in range(2):
                        GROUPS.append((p, br, hh, c))

        emap = {}
        st = {"stg": None, "et": None, "w": 0}

        # quantum scheduler state
        HIQ = deque()   # (cycles, fn) latency-sensitive: av, transp, outproj
        LOQ = []        # (deadline_chunk, earliest_chunk, cycles, fn), dl-sorted
        credit = [0.0]
        turn = ["lo"]

        def lo_pop_ready(cid):
            for i, (dl, est, cyc, fn) in enumerate(LOQ):
                if est <= cid:
                    LOQ.pop(i)
                    return cyc, fn
                if dl > cid + 40:
                    break
            return None

        def pump(budget, cid):
            credit[0] = min(credit[0] + budget, 8000.0)
            nlo = nhi = 0
            while credit[0] > 0 and (HIQ or LOQ) and nlo + nhi < 4:
                did = False
                order = ("lo", "hi") if turn[0] == "lo" else ("hi", "lo")
                for pref in order:
                    if pref == "lo" and LOQ and nlo < 1:
                        got = lo_pop_ready(cid)
                        if got is None:
                            continue
                        cyc, fn = got
                        turn[0] = "hi"
                        nlo += 1
                    elif pref == "hi" and HIQ and nhi < 2:
                        cyc, fn = HIQ.popleft()
                        turn[0] = "lo"
                        nhi += 1
                    else:
                        continue
                    fn()
                    credit[0] -= cyc
                    did = True
                    break
                if not did:
                    break

        def force_due(cid):
            # at most one forced pop per chunk so score matmuls interleave
            # and cover the single-bank drain latency
            if LOQ and LOQ[0][0] <= cid:
                _, _, cyc, fn = LOQ.pop(0)
                fn()
                credit[0] -= cyc

        def emit_chunk(cid):
            gi, lkt = divmod(cid, NLT)
            p, br, hh, c = GROUPS[gi]
            kT = kgT_sb if br == 0 else klT_sb
            if cid >= NCHUNK - 3:
                if cid == NCHUNK - 3 and st.get("fill", 0) > 0:
                    # flush the partially-filled staging tile
                    w = st["fill"] * 512
                    nc.scalar.activation(st["et"][:, :w], st["stg"][:, :w],
                                         mybir.ActivationFunctionType.Exp)
                    st["fill"] = 0
                tg = "a" if cid % 2 else "b"
                st["stg"] = stg.tile([128, 512], F32, tag=f"s{tg}", name=f"sf{cid}")
                st["et"] = et.tile([128, 512], BF16, tag=f"e{tg}", name=f"ef{cid}")
                st["w"] = 1
                slot = 0
            else:
                pos = cid % PAIR
                if pos == 0:
                    st["stg"] = stg.tile([128, WA], F32, tag="sa", name=f"sa{cid}")
                    st["et"] = et.tile([128, WA], BF16, tag="ea", name=f"ea{cid}")
                    st["w"] = CA
                elif pos == CA:
                    st["stg"] = stg.tile([128, WB], F32, tag="sb", name=f"sb{cid}")
                    st["et"] = et.tile([128, WB], BF16, tag="eb", name=f"eb{cid}")
                    st["w"] = CB
                slot = pos if pos < CA else pos - CA
            if _DBG:
                print(f"chunk {cid} (gi {gi} lkt {lkt})")
            force_due(cid)
            nc.tensor.matmul(
                st["stg"][:, slot * 512:slot * 512 + 512],
                kT[hh * 64:hh * 64 + 64, p, lkt * 128:lkt * 128 + 128],
                qT_sb[hh * 64:hh * 64 + 64, p, c * 512:c * 512 + 512],
                start=True, stop=True)
            emap[cid] = (st["et"], slot)
            st["fill"] = slot + 1
            if slot == st["w"] - 1 or cid == NCHUNK - 1:
                st["fill"] = 0
                w = (slot + 1) * 512
                nc.scalar.activation(st["et"][:, :w], st["stg"][:, :w],
                                     mybir.ActivationFunctionType.Exp)
                act_cyc = (w * 0.8333 + 185.0) / 0.41666
                pump(act_cyc - w, cid)

        # ---- LOW queue: projections, (deadline, earliest) in chunk units ----
        import bisect

        def lo(cyc, fn, dl, est=0):
            if _DBG:
                fn0 = fn

                def fn(fn0=fn0, dl=dl):
                    print(f"  LO pop dl={dl} {fn0}")
                    fn0()
            bisect.insort(LOQ, (dl, est, cyc, fn), key=lambda x: x[0])

        for qb in range(1, 4):
            lo(2048, (lambda qb=qb: proj_chunk(kgT_sb, wkg_sb, 0, qb, kx, off=2,
                                               ptag=("av" if qb % 2 else "pp"))),
               4 * qb, 0)
        for lt in range(16):
            lo(1024, (lambda lt=lt: v_chunk(lt)), 14 + 2 * lt, 12)
        for qb in range(1, 4):
            lo(2048, (lambda qb=qb: proj_chunk(qT_sb, wq_sb, 0, qb, xq,
                                               ptag=("av" if qb == 1 else "pp"))),
               32 * qb - 6, 18 if qb == 1 else 34)
        for i in range(20):
            qb, t = divmod(i, 5)
            if t == 0:
                s = {}
            lo(2048, (lambda s=s, qb=qb, t=t: klT_burst(s, 0, qb, t)),
               30 + 10 * i, 24)
        for qb in range(4):
            lo(2048, (lambda qb=qb: proj_chunk(qT_sb, wq_sb, 1, qb, xq,
                                               ptag=("av" if qb % 2 else "pp"))),
               120 + 32 * qb, 104 + 32 * qb)
        for qb in range(4):
            lo(2048, (lambda qb=qb: proj_chunk(kgT_sb, wkg_sb, 1, qb, kx, off=2,
                                               ptag=("av" if qb % 2 else "pp"))),
               70 + 6 * qb, 56 + 6 * qb)
        for i in range(20):
            qb, t = divmod(i, 5)
            if t == 0:
                s2 = {}
            lo(2048, (lambda s2=s2, qb=qb, t=t: klT_burst(s2, 1, qb, t)),
               250 + 5 * i, 150)

        # HIGH pushes happen at group completion: precompute per-group
        def push_av_group(gi):
            for sub in range(4):
                def avfn(gi=gi, sub=sub):
                    if _DBG:
                        print(f"  HI pop av({gi},{sub})")
                    av_sub(gi, sub)
                HIQ.append((1040, avfn))
            p, br, hh, c = GROUPS[gi]
            if br == 1 and hh == 1:
                for sub in range(4):
                    HIQ.append((300, lambda p=p, c=c, sub=sub: transp(p, c, sub)))
                if p == 1:
                    for lt in range(c * 4, c * 4 + 4):
                        HIQ.append((1024, lambda lt=lt: outproj(lt)))

        # av push lag in groups: 3 early (v DMA must land first), then 1,
        # then 0 at the end so the tail chain is minimal
        def av_push_gi(gi):
            return gi + 2 if gi < 8 else (gi + 1 if gi < 28 else gi)

        # ---- prologue projections (rest of kgT m0 arrives via LOW DLs) ----
        proj_chunk(kgT_sb, wkg_sb, 0, 0, kx, off=2)
        proj_chunk(qT_sb, wq_sb, 0, 0, xq)

        # ---- main stream ----
        pushed = set()
        for cid in range(NCHUNK):
            emit_chunk(cid)
            if cid % NLT == NLT - 1:
                gdone = cid // NLT
                for g in range(32):
                    if g not in pushed and av_push_gi(g) <= gdone:
                        push_av_group(g)
                        pushed.add(g)

        # ---- tail ----
        for g in range(32):
            if g not in pushed:
                push_av_group(g)
                pushed.add(g)
        while HIQ or LOQ:
            if LOQ:
                _, _, _, fn = LOQ.pop(0)
            else:
                _, fn = HIQ.popleft()
            fn()

    nc.compile()
    return nc


def _host_prep(inputs):
    """Fold conv+bn+biases; build the 8 per-core input maps."""
    f32 = np.float32
    q = np.ascontiguousarray(inputs["query"], dtype=f32)
    k = np.ascontiguousarray(inputs["key"], dtype=f32)
    v = np.ascontiguousarray(inputs["value"], dtype=f32)
    w3 = np.asarray(inputs["conv_w3"], f32)
    w5 = np.asarray(inputs["conv_w5"], f32)
    b3 = np.asarray(inputs["conv_b3"], f32)
    b5 = np.asarray(inputs["conv_b5"], f32)
    gam = np.asarray(inputs["bn_gamma"], f32)
    bet = np.asarray(inputs["bn_beta"], f32)
    mu = np.asarray(inputs["bn_mean"], f32)
    var = np.asarray(inputs["bn_var"], f32)
    wq = np.asarray(inputs["wq"], f32)
    bq = np.asarray(inputs["bq"], f32)
    wkl = np.asarray(inputs["wkl"], f32)
    bkl = np.asarray(inputs["bkl"], f32)
    wkg = np.asarray(inputs["wkg"], f32)
    bkg = np.asarray(inputs["bkg"], f32)
    wv = np.asarray(inputs["wv"], f32)
    bv = np.asarray(inputs["bv"], f32)
    wo = np.asarray(inputs["wo"], f32)
    bo = np.asarray(inputs["bo"], f32)

    assert not np.any(bq) and not np.any(bkg), "nonzero q/kg bias unsupported"

    s_bn = gam / np.sqrt(var + BN_EPS)
    shift = np.concatenate([b3, b5]) * s_bn + (bet - mu * s_bn)
    wkl_s = wkl * s_bn[None, :]
    A3 = np.einsum("oc,cit->oit", wkl_s[:, :D], w3)
    A5 = np.einsum("oc,cit->oit", wkl_s[:, D:], w5)
    W5c = A5.copy()
    W5c[:, :, 1:4] += A3
    bkl_eff = wkl @ shift + bkl
    wq_eff = wq / math.sqrt(DK)
    bo_eff = bo + wo @ (2.0 * bv)

    bf = BF16_NP
    ident = np.eye(128, dtype=bf)
    in_maps = []
    for c in range(N_CORES):
        b = c // 2
        hg = c % 2
        sel = slice(hg * DO, hg * DO + DO)
        wo_sel = wo.T[sel, :]
        wo2 = np.ascontiguousarray(
            wo_sel.reshape(2, 2, 64, D).transpose(1, 2, 0, 3).reshape(128, 2, D))
        in_maps.append({
            "xq": np.ascontiguousarray(q[b].T).astype(bf),
            "xk": np.ascontiguousarray(k[b].T).astype(bf),
            "xv": np.ascontiguousarray(v[b].T).astype(bf),
            "wq": np.ascontiguousarray(wq_eff.T[:, sel]).astype(bf),
            "wk5": np.ascontiguousarray(W5c.transpose(2, 1, 0)[:, :, sel]).astype(bf),
            "wkg": np.ascontiguousarray(wkg.T[:, sel]).astype(bf),
            "wv": np.ascontiguousarray(wv.T[:, sel]).astype(bf),
            "wo2": wo2.astype(bf),
            "bkl": np.ascontiguousarray(bkl_eff[sel]).astype(f32),
            "ident": ident,
        })
    return in_maps, bo_eff


def kernel(**inputs) -> np.ndarray:
    if "nc" not in _cache:
        _cache["nc"] = _build_program()
    nc = _cache["nc"]
    in_maps, bo_eff = _host_prep(inputs)
    res = bass_utils.run_bass_kernel_spmd(
        nc, in_maps, core_ids=list(range(N_CORES)))
    out = np.zeros((B, L, D), np.float32)
    for c in range(N_CORES):
        out[c // 2] += np.asarray(res.results[c]["out"], np.float32)
    out += bo_eff[None, None, :]
    return out


# revision 9
# speedup vs baseline: 1.0755x; 1.0046x over previous
"""Trainium2 Bass kernel v2 for nn_MultiHeadedAttention_4269197492266.

Same math as v1 (folded 5-tap conv local-key path, batch x head-group
sharding, ones-column softmax denominator), restructured around the
cost model:

- Score chunks stream continuously into [128,1536] PSUM staging tiles
  (two 3-bank buffers, ping-pong); exp runs as 171 wide ACT instructions (~250us, the
  ACT floor for 33.5M exps/core).  All other PE work (projections, conv
  taps, AV, transposes, outproj) is queued as sub-microsecond quanta and
  popped between staging tiles under a cycle budget with emission
  deadlines, so the in-order PE queue never starves ACT.
- AV is reoriented: out[lq, dk+1] = e_chunk^T @ V -- each matmul streams
  65 columns instead of 512, halving AV cost.  Group-major score order
  makes AV accumulators sequential: one PSUM bank suffices.
- The softmax denominator lands in column 64 per lq-partition:
  normalization is reciprocal + per-partition tensor_scalar on DVE.
- Branches combine before the output projection (scalar_tensor_tensor),
  halving outproj; x returns to [hdk, lq] via PE transposes (host-fed
  identity matrix).
"""

import math
import os
from collections import deque
from contextlib import ExitStack

_DBG = os.environ.get("KV2_DEBUG") == "1"

import ml_dtypes
import numpy as np

import concourse.tile as tile
from concourse import bacc, mybir
from concourse import bass_utils

F32 = mybir.dt.float32
BF16 = mybir.dt.bfloat16
BF16_NP = ml_dtypes.bfloat16

B, L, D = 4, 2048, 512
H, DK = 8, 64
N_CORES = 8
HG = 4
DO = HG * DK
BN_EPS = 1e-5
NJ = D // 128
NLT = L // 128
WA, WB = 1536, 1536          # alternating staging widths
CA, CB = WA // 512, WB // 512  # chunks per tile: 4, 2
PAIR = CA + CB               # 6 chunks per A/B pair
NCHUNK = 512                 # 2p * 2br * 2hh * 4c * 16lkt / ... = 32 groups * 16

_cache = {}


def _build_program():
    nc = bacc.Bacc("TRN2", target_bir_lowering=False, debug=False,
                   num_devices=N_CORES)

    dt_in = {}
    for nm in ("xq", "xk", "xv"):
        dt_in[nm] = nc.dram_tensor(nm, [D, L], BF16, kind="ExternalInput").ap()
    for nm in ("wq", "wkg", "wv"):
        dt_in[nm] = nc.dram_tensor(nm, [D, DO], BF16, kind="ExternalInput").ap()
    dt_in["wk5"] = nc.dram_tensor("wk5", [5, D, DO], BF16, kind="ExternalInput").ap()
    dt_in["wo2"] = nc.dram_tensor("wo2", [128, 2, D], BF16, kind="ExternalInput").ap()
    dt_in["bkl"] = nc.dram_tensor("bkl", [DO], F32, kind="ExternalInput").ap()
    dt_in["ident"] = nc.dram_tensor("ident", [128, 128], BF16, kind="ExternalInput").ap()
    out_ap = nc.dram_tensor("out", [L, D], BF16, kind="ExternalOutput").ap()

    with tile.TileContext(nc) as tc, ExitStack() as ctx:
        big = ctx.enter_context(tc.tile_pool(name="big", bufs=12))
        et = ctx.enter_context(tc.tile_pool(name="et", bufs=12))
        proj = ctx.enter_context(tc.tile_pool(name="projsb", bufs=1))
        norm = ctx.enter_context(tc.tile_pool(name="norm", bufs=8))
        ostage = ctx.enter_context(tc.tile_pool(name="ostage", bufs=3))
        stg = ctx.enter_context(tc.tile_pool(name="stg", bufs=1, space="PSUM"))
        wk = ctx.enter_context(tc.tile_pool(name="wk", bufs=1, space="PSUM"))

        # ---- persistent SBUF tensors ----
        wq_sb = proj.tile([128, NJ, DO], BF16, tag="wq")
        wk5_sb = proj.tile([128, 5, NJ, DO], BF16, tag="wk5")
        wkg_sb = proj.tile([128, NJ, DO], BF16, tag="wkg")
        wv_sb = proj.tile([128, NJ, DO], BF16, tag="wv")
        wo_sb = proj.tile([128, 2, D], BF16, tag="wo")
        id_sb = proj.tile([128, 128], BF16, tag="ident")
        bkl_sb = proj.tile([128, 2], F32, tag="bkl")
        qT_sb = proj.tile([128, 2, L], BF16, tag="qT")
        klT_sb = proj.tile([128, 2, L], BF16, tag="klT")
        kgT_sb = proj.tile([128, 2, L], BF16, tag="kgT")
        v_sb = proj.tile([128, NLT, HG, DK + 1], BF16, tag="v")
        x0_sb = proj.tile([128, 2, 2, 4, 4, DK], BF16, tag="x0")
        x1_sb = proj.tile([128, 2, 4, 4, DK], BF16, tag="x1")
        xT_sb = proj.tile([128, 2, NLT, 128], BF16, tag="xT")
        zw_sb = proj.tile([128, 512], BF16, tag="zw")

        # ---- input DMA: few big ops, 3 queues, need-by ordering ----
        LKP = L + 4
        kxall = big.tile([128, NJ, LKP], BF16, tag="kx", name="kxall", bufs=1)
        xqall = big.tile([128, NJ, L], BF16, tag="xq", name="xqall", bufs=1)
        xvall = big.tile([128, NJ, L], BF16, tag="xv", name="xvall", bufs=1)
        kx = [kxall[:, j, :] for j in range(NJ)]
        xq = [xqall[:, j, :] for j in range(NJ)]
        xv = [xvall[:, j, :] for j in range(NJ)]
        nc.vector.memset(kxall[:, :, 0:2], 0.0)
        nc.vector.memset(kxall[:, :, 2 + L:], 0.0)
        xk_r = dt_in["xk"].rearrange("(j p) l -> p j l", p=128)
        xq_r = dt_in["xq"].rearrange("(j p) l -> p j l", p=128)
        xv_r = dt_in["xv"].rearrange("(j p) l -> p j l", p=128)
        # The cost model serializes all DMA transfers on one resource, so
        # queue parallelism buys nothing: issue everything on one queue in
        # exact need order, sized so each lands just before its consumer.
        nc.sync.dma_start(kxall[:, :, 2:518], xk_r[:, :, 0:516])        # kgT qb0
        nc.sync.dma_start(wkg_sb[:], dt_in["wkg"].rearrange("(j p) o -> p j o", p=128))
        nc.sync.dma_start(wq_sb[:], dt_in["wq"].rearrange("(j p) o -> p j o", p=128))
        nc.sync.dma_start(xqall[:, :, 0:512], xq_r[:, :, 0:512])        # qT c0
        nc.sync.dma_start(kxall[:, :, 518:1034], xk_r[:, :, 516:1032])  # kgT qb1
        nc.sync.dma_start(kxall[:, :, 1034:2 + L], xk_r[:, :, 1032:])   # kgT qb2-3
        nc.sync.dma_start(xvall[:, 0:2, :], xv_r[:, 0:2, :])
        nc.sync.dma_start(wv_sb[:], dt_in["wv"].rearrange("(j p) o -> p j o", p=128))
        nc.sync.dma_start(xvall[:, 2:4, :], xv_r[:, 2:4, :])
        nc.sync.dma_start(xqall[:, :, 512:1024], xq_r[:, :, 512:1024])  # qT c1
        nc.sync.dma_start(wk5_sb[:], dt_in["wk5"].rearrange("t (j p) o -> p t j o", p=128))
        nc.sync.dma_start(xqall[:, :, 1024:], xq_r[:, :, 1024:])        # qT c2-3
        nc.sync.dma_start(bkl_sb[:], dt_in["bkl"].rearrange("(m p) -> p m", p=128))
        nc.sync.dma_start(wo_sb[:], dt_in["wo2"])
        nc.sync.dma_start(id_sb[:], dt_in["ident"])

        warm = proj.tile([1, 16], F32, tag="warmt")
        nc.vector.memset(warm[:], 0.0)
        nc.scalar.activation(warm[:], warm[:], mybir.ActivationFunctionType.Exp)

        # ---- PE warm-up: ramp the p-state while input DMA lands ----
        nc.vector.memset(zw_sb[:], 0.0)
        for i in range(12):
            zp = wk.tile([128, 512], F32, tag="pp", name=f"zp{i}")
            nc.tensor.matmul(zp[:], zw_sb[:, 0:128], zw_sb[:], start=True, stop=True)

        # ---- emitters ----
        def proj_chunk(dst_sb, w_sb, m, qb, src, bias=None, off=0, ptag="pp"):
            ps = wk.tile([128, 512], F32, tag=ptag, name=f"pp{m}_{qb}")
            for j in range(NJ):
                nc.tensor.matmul(ps[:], w_sb[:, j, m * 128:(m + 1) * 128],
                                 src[j][:, off + qb * 512:off + qb * 512 + 512],
                                 start=(j == 0), stop=(j == NJ - 1))
            if bias is not None:
                nc.vector.tensor_scalar_add(
                    dst_sb[:, m, qb * 512:qb * 512 + 512], ps[:], bias[:, m:m + 1])
            else:
                nc.vector.tensor_copy(dst_sb[:, m, qb * 512:qb * 512 + 512], ps[:])

        def klT_burst(state, m, qb, t):
            if t == 0:
                tg = "av" if (m == 0 and qb == 0) else "pp"
                state["ps"] = wk.tile([128, 512], F32, tag=tg, name=f"kl{m}_{qb}")
            ps = state["ps"]
            for j in range(NJ):
                nc.tensor.matmul(ps[:], wk5_sb[:, t, j, m * 128:(m + 1) * 128],
                                 kx[j][:, qb * 512 + t:qb * 512 + t + 512],
                                 start=(t == 0 and j == 0), stop=(t == 4 and j == NJ - 1))
            if t == 4:
                nc.vector.tensor_scalar_add(
                    klT_sb[:, m, qb * 512:qb * 512 + 512], ps[:], bkl_sb[:, m:m + 1])

        def v_chunk(lt):
            if lt == 0:
                nc.vector.memset(v_sb[:], 1.0)
            ps = wk.tile([128, 512], F32, tag=("pp" if lt % 2 == 0 else "av"),
                         name=f"vp{lt}")
            for j in range(NJ):
                nc.tensor.matmul(ps[:, :DO], xv[j][:, lt * 128:lt * 128 + 128],
                                 wv_sb[:, j, :],
                                 start=(j == 0), stop=(j == NJ - 1))
            nc.vector.tensor_copy(
                v_sb[:, lt, :, 0:DK],
                ps[:, :DO].rearrange("p (h d) -> p h d", h=HG))

        def av_sub(gi, sub):
            p, br, hh, c = GROUPS[gi]
            h = 2 * p + hh
            tag = ("pp" if (gi >= 28 and sub % 2 == 1) else "av")
            av = wk.tile([128, DK + 1], F32, tag=tag, name=f"av{gi}_{sub}")
            for lkt in range(NLT):
                e_t, slot = emap[gi * NLT + lkt]
                nc.tensor.matmul(
                    av[:],
                    e_t[:, slot * 512 + sub * 128:slot * 512 + sub * 128 + 128],
                    v_sb[:, lkt, h % HG, :],
                    start=(lkt == 0), stop=(lkt == NLT - 1))
            # one fast copy frees the PSUM bank; normalize from the copy so
            # the next av accumulation never waits on the norm round-trip
            avc = norm.tile([128, DK + 1], F32, tag="avc", name=f"avc{gi}_{sub}")
            nc.vector.tensor_copy(avc[:], av[:])
            rd = norm.tile([128, 1], F32, tag="rd", name=f"rd{gi}_{sub}")
            nc.vector.reciprocal(rd[:], avc[:, DK:DK + 1])
            if br == 0:
                nc.vector.tensor_scalar_mul(
                    x0_sb[:, p, hh, c, sub, :], avc[:, 0:DK], rd[:])
            else:
                nc.vector.scalar_tensor_tensor(
                    x1_sb[:, hh, c, sub, :], avc[:, 0:DK], rd[:],
                    x0_sb[:, p, hh, c, sub, :],
                    mybir.AluOpType.mult, mybir.AluOpType.add)

        def transp(p, c, sub):
            lt = c * 4 + sub
            tp = wk.tile([128, 128], BF16, tag="av", name=f"tp{p}_{lt}")
            for hh in range(2):
                nc.tensor.matmul(tp[hh * 64:hh * 64 + 64, :],
                                 x1_sb[:, hh, c, sub, :], id_sb[:],
                                 is_transpose=True)
            nc.vector.tensor_copy(xT_sb[:, p, lt, :], tp[:])

        def outproj(lt):
            po = wk.tile([128, 512], F32, tag=("pp" if lt % 2 == 0 else "av"),
                         name=f"po{lt}")
            for p in range(2):
                nc.tensor.matmul(po[:], xT_sb[:, p, lt, :], wo_sb[:, p, :],
                                 start=(p == 0), stop=(p == 1))
            ot = ostage.tile([128, D], BF16, tag="ot", name=f"ot{lt}")
            nc.vector.tensor_copy(ot[:], po[:])
            nc.sync.dma_start(out_ap[lt * 128:lt * 128 + 128, :], ot[:])

        # ---- group sequence and chunk stream ----
        GROUPS = []
        for br in range(2):
            for p in range(2):
                for c in range(4):
                    for hh in range(2):
                        GROUPS.append((p, br, hh, c))

        emap = {}
        st = {"stg": None, "et": None, "w": 0}

        # quantum scheduler state
        HIQ = deque()   # (cycles, fn) latency-sensitive: av, transp, outproj
        LOQ = []        # (deadline_chunk, earliest_chunk, cycles, fn), dl-sorted
        credit = [0.0]
        turn = ["lo"]

        def lo_pop_ready(cid):
            for i, (dl, est, cyc, fn) in enumerate(LOQ):
                if est <= cid:
                    LOQ.pop(i)
                    return cyc, fn
                if dl > cid + 40:
                    break
            return None

        def pump(budget, cid):
            credit[0] = min(credit[0] + budget, 8000.0)
            nlo = nhi = 0
            while credit[0] > 0 and (HIQ or LOQ) and nlo + nhi < 4:
                did = False
                order = ("lo", "hi") if turn[0] == "lo" else ("hi", "lo")
                for pref in order:
                    if pref == "lo" and LOQ and nlo < 1:
                        got = lo_pop_ready(cid)
                        if got is None:
                            continue
                        cyc, fn = got
                        turn[0] = "hi"
                        nlo += 1
                    elif pref == "hi" and HIQ and nhi < 2:
                        cyc, fn = HIQ.popleft()
                        turn[0] = "lo"
                        nhi += 1
                    else:
                        continue
                    fn()
                    credit[0] -= cyc
                    did = True
                    break
                if not did:
                    break

        def force_due(cid):
            # at most one forced pop per chunk so score matmuls interleave
            # and cover the single-bank drain latency
            if LOQ and LOQ[0][0] <= cid:
                _, _, cyc, fn = LOQ.pop(0)
                fn()
                credit[0] -= cyc

        def emit_chunk(cid):
            gi, lkt = divmod(cid, NLT)
            p, br, hh, c = GROUPS[gi]
            kT = kgT_sb if br == 0 else klT_sb
            if cid >= NCHUNK - 3:
                if cid == NCHUNK - 3 and st.get("fill", 0) > 0:
                    # flush the partially-filled staging tile
                    w = st["fill"] * 512
                    nc.scalar.activation(st["et"][:, :w], st["stg"][:, :w],
                                         mybir.ActivationFunctionType.Exp)
                    st["fill"] = 0
                tg = "a" if cid % 2 else "b"
                st["stg"] = stg.tile([128, 512], F32, tag=f"s{tg}", name=f"sf{cid}")
                st["et"] = et.tile([128, 512], BF16, tag=f"e{tg}", name=f"ef{cid}")
                st["w"] = 1
                slot = 0
            else:
                pos = cid % PAIR
                if pos == 0:
                    st["stg"] = stg.tile([128, WA], F32, tag="sa", name=f"sa{cid}")
                    st["et"] = et.tile([128, WA], BF16, tag="ea", name=f"ea{cid}")
                    st["w"] = CA
                elif pos == CA:
                    st["stg"] = stg.tile([128, WB], F32, tag="sb", name=f"sb{cid}")
                    st["et"] = et.tile([128, WB], BF16, tag="eb", name=f"eb{cid}")
                    st["w"] = CB
                slot = pos if pos < CA else pos - CA
            if _DBG:
                print(f"chunk {cid} (gi {gi} lkt {lkt})")
            force_due(cid)
            nc.tensor.matmul(
                st["stg"][:, slot * 512:slot * 512 + 512],
                kT[hh * 64:hh * 64 + 64, p, lkt * 128:lkt * 128 + 128],
                qT_sb[hh * 64:hh * 64 + 64, p, c * 512:c * 512 + 512],
                start=True, stop=True)
            emap[cid] = (st["et"], slot)
            st["fill"] = slot + 1
            if slot == st["w"] - 1 or cid == NCHUNK - 1:
                st["fill"] = 0
                w = (slot + 1) * 512
                nc.scalar.activation(st["et"][:, :w], st["stg"][:, :w],
                                     mybir.ActivationFunctionType.Exp)
                act_cyc = (w * 0.8333 + 185.0) / 0.41666
                pump(act_cyc - w, cid)

        # ---- LOW queue: projections, (deadline, earliest) in chunk units ----
        import bisect

        def lo(cyc, fn, dl, est=0):
            if _DBG:
                fn0 = fn

                def fn(fn0=fn0, dl=dl):
                    print(f"  LO pop dl={dl} {fn0}")
                    fn0()
            bisect.insort(LOQ, (dl, est, cyc, fn), key=lambda x: x[0])

        for qb in range(1, 4):
            lo(2048, (lambda qb=qb: proj_chunk(kgT_sb, wkg_sb, 0, qb, kx, off=2,
                                               ptag=("av" if qb % 2 else "pp"))),
               4 * qb, 0)
        for lt in range(16):
            lo(1024, (lambda lt=lt: v_chunk(lt)), 14 + 2 * lt, 12)
        for qb in range(1, 4):
            lo(2048, (lambda qb=qb: proj_chunk(qT_sb, wq_sb, 0, qb, xq,
                                               ptag=("av" if qb == 1 else "pp"))),
               32 * qb - 6, 18 if qb == 1 else 34)
        for i in range(20):
            qb, t = divmod(i, 5)
            if t == 0:
                s = {}
            lo(2048, (lambda s=s, qb=qb, t=t: klT_burst(s, 0, qb, t)),
               30 + 10 * i, 24)
        for qb in range(4):
            lo(2048, (lambda qb=qb: proj_chunk(qT_sb, wq_sb, 1, qb, xq,
                                               ptag=("av" if qb % 2 else "pp"))),
               120 + 32 * qb, 104 + 32 * qb)
        for qb in range(4):
            lo(2048, (lambda qb=qb: proj_chunk(kgT_sb, wkg_sb, 1, qb, kx, off=2,
                                               ptag=("av" if qb % 2 else "pp"))),
               70 + 6 * qb, 56 + 6 * qb)
        for i in range(20):
            qb, t = divmod(i, 5)
            if t == 0:
                s2 = {}
            lo(2048, (lambda s2=s2, qb=qb, t=t: klT_burst(s2, 1, qb, t)),
               250 + 5 * i, 236 + 5 * i)

        # HIGH pushes happen at group completion: precompute per-group
        def push_av_group(gi):
            for sub in range(4):
                def avfn(gi=gi, sub=sub):
                    if _DBG:
                        print(f"  HI pop av({gi},{sub})")
                    av_sub(gi, sub)
                HIQ.append((1040, avfn))
            p, br, hh, c = GROUPS[gi]
            if br == 1 and hh == 1:
                for sub in range(4):
                    HIQ.append((300, lambda p=p, c=c, sub=sub: transp(p, c, sub)))
                if p == 1:
                    for lt in range(c * 4, c * 4 + 4):
                        HIQ.append((1024, lambda lt=lt: outproj(lt)))

        # av push lag in groups: 3 early (v DMA must land first), then 1,
        # then 0 at the end so the tail chain is minimal
        def av_push_gi(gi):
            return gi + 2 if gi < 8 else (gi + 1 if gi < 28 else gi)

        # ---- prologue projections (rest of kgT m0 arrives via LOW DLs) ----
        proj_chunk(kgT_sb, wkg_sb, 0, 0, kx, off=2)
        proj_chunk(qT_sb, wq_sb, 0, 0, xq)

        # ---- main stream ----
        pushed = set()
        for cid in range(NCHUNK):
            emit_chunk(cid)
            if cid % NLT == NLT - 1:
                gdone = cid // NLT
                for g in range(32):
                    if g not in pushed and av_push_gi(g) <= gdone:
                        push_av_group(g)
                        pushed.add(g)

        # ---- tail ----
        for g in range(32):
            if g not in pushed:
                push_av_group(g)
                pushed.add(g)
        while HIQ or LOQ:
            if LOQ:
                _, _, _, fn = LOQ.pop(0)
            else:
                _, fn = HIQ.popleft()
            fn()

    nc.compile()
    return nc


def _host_prep(inputs):
    """Fold conv+bn+biases; build the 8 per-core input maps."""
    f32 = np.float32
    q = np.ascontiguousarray(inputs["query"], dtype=f32)
    k = np.ascontiguousarray(inputs["key"], dtype=f32)
    v = np.ascontiguousarray(inputs["value"], dtype=f32)
    w3 = np.asarray(inputs["conv_w3"], f32)
    w5 = np.asarray(inputs["conv_w5"], f32)
    b3 = np.asarray(inputs["conv_b3"], f32)
    b5 = np.asarray(inputs["conv_b5"], f32)
    gam = np.asarray(inputs["bn_gamma"], f32)
    bet = np.asarray(inputs["bn_beta"], f32)
    mu = np.asarray(inputs["bn_mean"], f32)
    var = np.asarray(inputs["bn_var"], f32)
    wq = np.asarray(inputs["wq"], f32)
    bq = np.asarray(inputs["bq"], f32)
    wkl = np.asarray(inputs["wkl"], f32)
    bkl = np.asarray(inputs["bkl"], f32)
    wkg = np.asarray(inputs["wkg"], f32)
    bkg = np.asarray(inputs["bkg"], f32)
    wv = np.asarray(inputs["wv"], f32)
    bv = np.asarray(inputs["bv"], f32)
    wo = np.asarray(inputs["wo"], f32)
    bo = np.asarray(inputs["bo"], f32)

    assert not np.any(bq) and not np.any(bkg), "nonzero q/kg bias unsupported"

    s_bn = gam / np.sqrt(var + BN_EPS)
    shift = np.concatenate([b3, b5]) * s_bn + (bet - mu * s_bn)
    wkl_s = wkl * s_bn[None, :]
    A3 = np.einsum("oc,cit->oit", wkl_s[:, :D], w3)
    A5 = np.einsum("oc,cit->oit", wkl_s[:, D:], w5)
    W5c = A5.copy()
    W5c[:, :, 1:4] += A3
    bkl_eff = wkl @ shift + bkl
    wq_eff = wq / math.sqrt(DK)
    bo_eff = bo + wo @ (2.0 * bv)

    bf = BF16_NP
    ident = np.eye(128, dtype=bf)
    in_maps = []
    for c in range(N_CORES):
        b = c // 2
        hg = c % 2
        sel = slice(hg * DO, hg * DO + DO)
        wo_sel = wo.T[sel, :]
        wo2 = np.ascontiguousarray(
            wo_sel.reshape(2, 2, 64, D).transpose(1, 2, 0, 3).reshape(128, 2, D))
        in_maps.append({
            "xq": np.ascontiguousarray(q[b].T).astype(bf),
            "xk": np.ascontiguousarray(k[b].T).astype(bf),
            "xv": np.ascontiguousarray(v[b].T).astype(bf),
            "wq": np.ascontiguousarray(wq_eff.T[:, sel]).astype(bf),
            "wk5": np.ascontiguousarray(W5c.transpose(2, 1, 0)[:, :, sel]).astype(bf),
            "wkg": np.ascontiguousarray(wkg.T[:, sel]).astype(bf),
            "wv": np.ascontiguousarray(wv.T[:, sel]).astype(bf),
            "wo2": wo2.astype(bf),
            "bkl": np.ascontiguousarray(bkl_eff[sel]).astype(f32),
            "ident": ident,
        })
    return in_maps, bo_eff


def kernel(**inputs) -> np.ndarray:
    if "nc" not in _cache:
        _cache["nc"] = _build_program()
    nc = _cache["nc"]
    in_maps, bo_eff = _host_prep(inputs)
    res = bass_utils.run_bass_kernel_spmd(
        nc, in_maps, core_ids=list(range(N_CORES)))
    out = np.zeros((B, L, D), np.float32)
    for c in range(N_CORES):
        out[c // 2] += np.asarray(res.results[c]["out"], np.float32)
    out += bo_eff[None, None, :]
    return out


# revision 10
# speedup vs baseline: 1.0813x; 1.0054x over previous
"""Trainium2 Bass kernel v2 for nn_MultiHeadedAttention_4269197492266.

Same math as v1 (folded 5-tap conv local-key path, batch x head-group
sharding, ones-column softmax denominator), restructured around the
cost model:

- Score chunks stream continuously into [128,1536] PSUM staging tiles
  (two 3-bank buffers, ping-pong); exp runs as 171 wide ACT instructions (~250us, the
  ACT floor for 33.5M exps/core).  All other PE work (projections, conv
  taps, AV, transposes, outproj) is queued as sub-microsecond quanta and
  popped between staging tiles under a cycle budget with emission
  deadlines, so the in-order PE queue never starves ACT.
- AV is reoriented: out[lq, dk+1] = e_chunk^T @ V -- each matmul streams
  65 columns instead of 512, halving AV cost.  Group-major score order
  makes AV accumulators sequential: one PSUM bank suffices.
- The softmax denominator lands in column 64 per lq-partition:
  normalization is reciprocal + per-partition tensor_scalar on DVE.
- Branches combine before the output projection (scalar_tensor_tensor),
  halving outproj; x returns to [hdk, lq] via PE transposes (host-fed
  identity matrix).
"""

import math
import os
from collections import deque
from contextlib import ExitStack

_DBG = os.environ.get("KV2_DEBUG") == "1"

import ml_dtypes
import numpy as np

import concourse.tile as tile
from concourse import bacc, mybir
from concourse import bass_utils

F32 = mybir.dt.float32
BF16 = mybir.dt.bfloat16
BF16_NP = ml_dtypes.bfloat16

B, L, D = 4, 2048, 512
H, DK = 8, 64
N_CORES = 8
HG = 4
DO = HG * DK
BN_EPS = 1e-5
NJ = D // 128
NLT = L // 128
WA, WB = 1536, 1536          # alternating staging widths
CA, CB = WA // 512, WB // 512  # chunks per tile: 4, 2
PAIR = CA + CB               # 6 chunks per A/B pair
NCHUNK = 512                 # 2p * 2br * 2hh * 4c * 16lkt / ... = 32 groups * 16

_cache = {}


def _build_program():
    nc = bacc.Bacc("TRN2", target_bir_lowering=False, debug=False,
                   num_devices=N_CORES)

    dt_in = {}
    for nm in ("xq", "xk", "xv"):
        dt_in[nm] = nc.dram_tensor(nm, [D, L], BF16, kind="ExternalInput").ap()
    for nm in ("wq", "wkg", "wv"):
        dt_in[nm] = nc.dram_tensor(nm, [D, DO], BF16, kind="ExternalInput").ap()
    dt_in["wk5"] = nc.dram_tensor("wk5", [5, D, DO], BF16, kind="ExternalInput").ap()
    dt_in["wo2"] = nc.dram_tensor("wo2", [128, 2, D], BF16, kind="ExternalInput").ap()
    dt_in["bkl"] = nc.dram_tensor("bkl", [DO], F32, kind="ExternalInput").ap()
    dt_in["ident"] = nc.dram_tensor("ident", [128, 128], BF16, kind="ExternalInput").ap()
    out_ap = nc.dram_tensor("out", [L, D], BF16, kind="ExternalOutput").ap()

    with tile.TileContext(nc) as tc, ExitStack() as ctx:
        big = ctx.enter_context(tc.tile_pool(name="big", bufs=12))
        et = ctx.enter_context(tc.tile_pool(name="et", bufs=12))
        proj = ctx.enter_context(tc.tile_pool(name="projsb", bufs=1))
        norm = ctx.enter_context(tc.tile_pool(name="norm", bufs=8))
        ostage = ctx.enter_context(tc.tile_pool(name="ostage", bufs=3))
        stg = ctx.enter_context(tc.tile_pool(name="stg", bufs=1, space="PSUM"))
        wk = ctx.enter_context(tc.tile_pool(name="wk", bufs=1, space="PSUM"))

        # ---- persistent SBUF tensors ----
        wq_sb = proj.tile([128, NJ, DO], BF16, tag="wq")
        wk5_sb = proj.tile([128, 5, NJ, DO], BF16, tag="wk5")
        wkg_sb = proj.tile([128, NJ, DO], BF16, tag="wkg")
        wv_sb = proj.tile([128, NJ, DO], BF16, tag="wv")
        wo_sb = proj.tile([128, 2, D], BF16, tag="wo")
        id_sb = proj.tile([128, 128], BF16, tag="ident")
        bkl_sb = proj.tile([128, 2], F32, tag="bkl")
        qT_sb = proj.tile([128, 2, L], BF16, tag="qT")
        klT_sb = proj.tile([128, 2, L], BF16, tag="klT")
        kgT_sb = proj.tile([128, 2, L], BF16, tag="kgT")
        v_sb = proj.tile([128, NLT, HG, DK + 1], BF16, tag="v")
        x0_sb = proj.tile([128, 2, 2, 4, 4, DK], BF16, tag="x0")
        x1_sb = proj.tile([128, 2, 4, 4, DK], BF16, tag="x1")
        xT_sb = proj.tile([128, 2, NLT, 128], BF16, tag="xT")
        zw_sb = proj.tile([128, 512], BF16, tag="zw")

        # ---- input DMA: few big ops, 3 queues, need-by ordering ----
        LKP = L + 4
        kxall = big.tile([128, NJ, LKP], BF16, tag="kx", name="kxall", bufs=1)
        xqall = big.tile([128, NJ, L], BF16, tag="xq", name="xqall", bufs=1)
        xvall = big.tile([128, NJ, L], BF16, tag="xv", name="xvall", bufs=1)
        kx = [kxall[:, j, :] for j in range(NJ)]
        xq = [xqall[:, j, :] for j in range(NJ)]
        xv = [xvall[:, j, :] for j in range(NJ)]
        nc.vector.memset(kxall[:, :, 0:2], 0.0)
        nc.vector.memset(kxall[:, :, 2 + L:], 0.0)
        xk_r = dt_in["xk"].rearrange("(j p) l -> p j l", p=128)
        xq_r = dt_in["xq"].rearrange("(j p) l -> p j l", p=128)
        xv_r = dt_in["xv"].rearrange("(j p) l -> p j l", p=128)
        # The cost model serializes all DMA transfers on one resource, so
        # queue parallelism buys nothing: issue everything on one queue in
        # exact need order, sized so each lands just before its consumer.
        nc.sync.dma_start(kxall[:, :, 2:518], xk_r[:, :, 0:516])        # kgT qb0
        nc.sync.dma_start(wkg_sb[:], dt_in["wkg"].rearrange("(j p) o -> p j o", p=128))
        nc.sync.dma_start(wq_sb[:], dt_in["wq"].rearrange("(j p) o -> p j o", p=128))
        nc.sync.dma_start(xqall[:, :, 0:512], xq_r[:, :, 0:512])        # qT c0
        nc.sync.dma_start(kxall[:, :, 518:1034], xk_r[:, :, 516:1032])  # kgT qb1
        nc.sync.dma_start(kxall[:, :, 1034:2 + L], xk_r[:, :, 1032:])   # kgT qb2-3
        nc.sync.dma_start(xvall[:, 0:2, :], xv_r[:, 0:2, :])
        nc.sync.dma_start(wv_sb[:], dt_in["wv"].rearrange("(j p) o -> p j o", p=128))
        nc.sync.dma_start(xvall[:, 2:4, :], xv_r[:, 2:4, :])
        nc.sync.dma_start(xqall[:, :, 512:1024], xq_r[:, :, 512:1024])  # qT c1
        nc.sync.dma_start(wk5_sb[:], dt_in["wk5"].rearrange("t (j p) o -> p t j o", p=128))
        nc.sync.dma_start(xqall[:, :, 1024:], xq_r[:, :, 1024:])        # qT c2-3
        nc.sync.dma_start(bkl_sb[:], dt_in["bkl"].rearrange("(m p) -> p m", p=128))
        nc.sync.dma_start(wo_sb[:], dt_in["wo2"])
        nc.sync.dma_start(id_sb[:], dt_in["ident"])

        warm = proj.tile([1, 16], F32, tag="warmt")
        nc.vector.memset(warm[:], 0.0)
        nc.scalar.activation(warm[:], warm[:], mybir.ActivationFunctionType.Exp)

        # ---- PE warm-up: ramp the p-state while input DMA lands ----
        nc.vector.memset(zw_sb[:], 0.0)
        for i in range(12):
            zp = wk.tile([128, 512], F32, tag="pp", name=f"zp{i}")
            nc.tensor.matmul(zp[:], zw_sb[:, 0:128], zw_sb[:], start=True, stop=True)

        # ---- emitters ----
        def proj_chunk(dst_sb, w_sb, m, qb, src, bias=None, off=0, ptag="pp"):
            ps = wk.tile([128, 512], F32, tag=ptag, name=f"pp{m}_{qb}")
            for j in range(NJ):
                nc.tensor.matmul(ps[:], w_sb[:, j, m * 128:(m + 1) * 128],
                                 src[j][:, off + qb * 512:off + qb * 512 + 512],
                                 start=(j == 0), stop=(j == NJ - 1))
            if bias is not None:
                nc.vector.tensor_scalar_add(
                    dst_sb[:, m, qb * 512:qb * 512 + 512], ps[:], bias[:, m:m + 1])
            else:
                nc.vector.tensor_copy(dst_sb[:, m, qb * 512:qb * 512 + 512], ps[:])

        def klT_burst(state, m, qb, t):
            if t == 0:
                tg = "av" if (m == 0 and qb == 0) else "pp"
                state["ps"] = wk.tile([128, 512], F32, tag=tg, name=f"kl{m}_{qb}")
            ps = state["ps"]
            for j in range(NJ):
                nc.tensor.matmul(ps[:], wk5_sb[:, t, j, m * 128:(m + 1) * 128],
                                 kx[j][:, qb * 512 + t:qb * 512 + t + 512],
                                 start=(t == 0 and j == 0), stop=(t == 4 and j == NJ - 1))
            if t == 4:
                nc.vector.tensor_scalar_add(
                    klT_sb[:, m, qb * 512:qb * 512 + 512], ps[:], bkl_sb[:, m:m + 1])

        def v_chunk(lt):
            if lt == 0:
                nc.vector.memset(v_sb[:], 1.0)
            ps = wk.tile([128, 512], F32, tag=("pp" if lt % 2 == 0 else "av"),
                         name=f"vp{lt}")
            for j in range(NJ):
                nc.tensor.matmul(ps[:, :DO], xv[j][:, lt * 128:lt * 128 + 128],
                                 wv_sb[:, j, :],
                                 start=(j == 0), stop=(j == NJ - 1))
            nc.vector.tensor_copy(
                v_sb[:, lt, :, 0:DK],
                ps[:, :DO].rearrange("p (h d) -> p h d", h=HG))

        def av_sub(gi, sub):
            p, br, hh, c = GROUPS[gi]
            h = 2 * p + hh
            tag = ("pp" if (gi >= 28 and sub % 2 == 1) else "av")
            av = wk.tile([128, DK + 1], F32, tag=tag, name=f"av{gi}_{sub}")
            for lkt in range(NLT):
                e_t, slot = emap[gi * NLT + lkt]
                nc.tensor.matmul(
                    av[:],
                    e_t[:, slot * 512 + sub * 128:slot * 512 + sub * 128 + 128],
                    v_sb[:, lkt, h % HG, :],
                    start=(lkt == 0), stop=(lkt == NLT - 1))
            # one fast copy frees the PSUM bank; normalize from the copy so
            # the next av accumulation never waits on the norm round-trip
            avc = norm.tile([128, DK + 1], F32, tag="avc", name=f"avc{gi}_{sub}")
            nc.vector.tensor_copy(avc[:], av[:])
            rd = norm.tile([128, 1], F32, tag="rd", name=f"rd{gi}_{sub}")
            nc.vector.reciprocal(rd[:], avc[:, DK:DK + 1])
            if br == 0:
                nc.vector.tensor_scalar_mul(
                    x0_sb[:, p, hh, c, sub, :], avc[:, 0:DK], rd[:])
            else:
                nc.vector.scalar_tensor_tensor(
                    x1_sb[:, hh, c, sub, :], avc[:, 0:DK], rd[:],
                    x0_sb[:, p, hh, c, sub, :],
                    mybir.AluOpType.mult, mybir.AluOpType.add)

        def transp(p, c, sub):
            lt = c * 4 + sub
            tp = wk.tile([128, 128], BF16, tag="av", name=f"tp{p}_{lt}")
            for hh in range(2):
                nc.tensor.matmul(tp[hh * 64:hh * 64 + 64, :],
                                 x1_sb[:, hh, c, sub, :], id_sb[:],
                                 is_transpose=True)
            nc.vector.tensor_copy(xT_sb[:, p, lt, :], tp[:])

        def outproj(lt):
            po = wk.tile([128, 512], F32, tag=("pp" if lt % 2 == 0 else "av"),
                         name=f"po{lt}")
            for p in range(2):
                nc.tensor.matmul(po[:], xT_sb[:, p, lt, :], wo_sb[:, p, :],
                                 start=(p == 0), stop=(p == 1))
            ot = ostage.tile([128, D], BF16, tag="ot", name=f"ot{lt}")
            nc.vector.tensor_copy(ot[:], po[:])
            nc.sync.dma_start(out_ap[lt * 128:lt * 128 + 128, :], ot[:])

        # ---- group sequence and chunk stream ----
        GROUPS = []
        for br in range(2):
            for p in range(2):
                for c in range(4):
                    for hh in range(2):
                        GROUPS.append((p, br, hh, c))

        emap = {}
        st = {"stg": None, "et": None, "w": 0}

        # quantum scheduler state
        HIQ = deque()   # (cycles, fn) latency-sensitive: av, transp, outproj
        LOQ = []        # (deadline_chunk, earliest_chunk, cycles, fn), dl-sorted
        credit = [0.0]
        turn = ["lo"]

        def lo_pop_ready(cid):
            for i, (dl, est, cyc, fn) in enumerate(LOQ):
                if est <= cid:
                    LOQ.pop(i)
                    return cyc, fn
                if dl > cid + 40:
                    break
            return None

        def pump(budget, cid):
            credit[0] = min(credit[0] + budget, 8000.0)
            nlo = nhi = 0
            while credit[0] > 0 and (HIQ or LOQ) and nlo + nhi < 4:
                did = False
                order = ("lo", "hi") if turn[0] == "lo" else ("hi", "lo")
                for pref in order:
                    if pref == "lo" and LOQ and nlo < 1:
                        got = lo_pop_ready(cid)
                        if got is None:
                            continue
                        cyc, fn = got
                        turn[0] = "hi"
                        nlo += 1
                    elif pref == "hi" and HIQ and nhi < 2:
                        cyc, fn = HIQ.popleft()
                        turn[0] = "lo"
                        nhi += 1
                    else:
                        continue
                    fn()
                    credit[0] -= cyc
                    did = True
                    break
                if not did:
                    break

        def force_due(cid):
            # at most one forced pop per chunk so score matmuls interleave
            # and cover the single-bank drain latency
            if LOQ and LOQ[0][0] <= cid:
                _, _, cyc, fn = LOQ.pop(0)
                fn()
                credit[0] -= cyc

        def emit_chunk(cid):
            gi, lkt = divmod(cid, NLT)
            p, br, hh, c = GROUPS[gi]
            kT = kgT_sb if br == 0 else klT_sb
            if cid >= NCHUNK - 3:
                if cid == NCHUNK - 3 and st.get("fill", 0) > 0:
                    # flush the partially-filled staging tile
                    w = st["fill"] * 512
                    nc.scalar.activation(st["et"][:, :w], st["stg"][:, :w],
                                         mybir.ActivationFunctionType.Exp)
                    st["fill"] = 0
                tg = "a" if cid % 2 else "b"
                st["stg"] = stg.tile([128, 512], F32, tag=f"s{tg}", name=f"sf{cid}")
                st["et"] = et.tile([128, 512], BF16, tag=f"e{tg}", name=f"ef{cid}")
                st["w"] = 1
                slot = 0
            else:
                pos = cid % PAIR
                if pos == 0:
                    st["stg"] = stg.tile([128, WA], F32, tag="sa", name=f"sa{cid}")
                    st["et"] = et.tile([128, WA], BF16, tag="ea", name=f"ea{cid}")
                    st["w"] = CA
                elif pos == CA:
                    st["stg"] = stg.tile([128, WB], F32, tag="sb", name=f"sb{cid}")
                    st["et"] = et.tile([128, WB], BF16, tag="eb", name=f"eb{cid}")
                    st["w"] = CB
                slot = pos if pos < CA else pos - CA
            if _DBG:
                print(f"chunk {cid} (gi {gi} lkt {lkt})")
            force_due(cid)
            nc.tensor.matmul(
                st["stg"][:, slot * 512:slot * 512 + 512],
                kT[hh * 64:hh * 64 + 64, p, lkt * 128:lkt * 128 + 128],
                qT_sb[hh * 64:hh * 64 + 64, p, c * 512:c * 512 + 512],
                start=True, stop=True)
            emap[cid] = (st["et"], slot)
            st["fill"] = slot + 1
            if slot == st["w"] - 1 or cid == NCHUNK - 1:
                st["fill"] = 0
                w = (slot + 1) * 512
                nc.scalar.activation(st["et"][:, :w], st["stg"][:, :w],
                                     mybir.ActivationFunctionType.Exp)
                act_cyc = (w * 0.8333 + 185.0) / 0.41666
                pump(act_cyc - w, cid)

        # ---- LOW queue: projections, (deadline, earliest) in chunk units ----
        import bisect

        def lo(cyc, fn, dl, est=0):
            if _DBG:
                fn0 = fn

                def fn(fn0=fn0, dl=dl):
                    print(f"  LO pop dl={dl} {fn0}")
                    fn0()
            bisect.insort(LOQ, (dl, est, cyc, fn), key=lambda x: x[0])

        for qb in range(1, 4):
            lo(2048, (lambda qb=qb: proj_chunk(kgT_sb, wkg_sb, 0, qb, kx, off=2,
                                               ptag=("av" if qb % 2 else "pp"))),
               4 * qb, 0)
        for lt in range(16):
            lo(1024, (lambda lt=lt: v_chunk(lt)), 14 + 2 * lt, 12)
        for qb in range(1, 4):
            lo(2048, (lambda qb=qb: proj_chunk(qT_sb, wq_sb, 0, qb, xq,
                                               ptag=("av" if qb == 1 else "pp"))),
               32 * qb - 6, 18 if qb == 1 else 34)
        for i in range(20):
            qb, t = divmod(i, 5)
            if t == 0:
                s = {}
            lo(2048, (lambda s=s, qb=qb, t=t: klT_burst(s, 0, qb, t)),
               30 + 10 * i, 24)
        for qb in range(4):
            lo(2048, (lambda qb=qb: proj_chunk(qT_sb, wq_sb, 1, qb, xq,
                                               ptag=("av" if qb % 2 else "pp"))),
               120 + 32 * qb, 104 + 32 * qb)
        for qb in range(4):
            lo(2048, (lambda qb=qb: proj_chunk(kgT_sb, wkg_sb, 1, qb, kx, off=2,
                                               ptag=("av" if qb % 2 else "pp"))),
               70 + 6 * qb, 56 + 6 * qb)
        for i in range(20):
            qb, t = divmod(i, 5)
            if t == 0:
                s2 = {}
            lo(2048, (lambda s2=s2, qb=qb, t=t: klT_burst(s2, 1, qb, t)),
               250 + 5 * i, 236 + 5 * i)

        # HIGH pushes happen at group completion: precompute per-group
        def push_av_group(gi):
            for sub in range(4):
                def avfn(gi=gi, sub=sub):
                    if _DBG:
                        print(f"  HI pop av({gi},{sub})")
                    av_sub(gi, sub)
                HIQ.append((1040, avfn))
            p, br, hh, c = GROUPS[gi]
            if br == 1 and hh == 1:
                for sub in range(4):
                    HIQ.append((300, lambda p=p, c=c, sub=sub: transp(p, c, sub)))
                if p == 1:
                    for lt in range(c * 4, c * 4 + 4):
                        HIQ.append((1024, lambda lt=lt: outproj(lt)))

        # av push lag in groups: 3 early (v DMA must land first), then 1,
        # then 0 at the end so the tail chain is minimal
        def av_push_gi(gi):
            if gi < 8:
                return gi + 2
            if gi < 16:
                return gi + 2
            return gi + 1 if gi < 28 else gi

        # ---- prologue projections (rest of kgT m0 arrives via LOW DLs) ----
        proj_chunk(kgT_sb, wkg_sb, 0, 0, kx, off=2)
        proj_chunk(qT_sb, wq_sb, 0, 0, xq)

        # ---- main stream ----
        pushed = set()
        for cid in range(NCHUNK):
            emit_chunk(cid)
            if cid % NLT == NLT - 1:
                gdone = cid // NLT
                for g in range(32):
                    if g not in pushed and av_push_gi(g) <= gdone:
                        push_av_group(g)
                        pushed.add(g)

        # ---- tail ----
        for g in range(32):
            if g not in pushed:
                push_av_group(g)
                pushed.add(g)
        while HIQ or LOQ:
            if LOQ:
                _, _, _, fn = LOQ.pop(0)
            else:
                _, fn = HIQ.popleft()
            fn()

    nc.compile()
    return nc


def _host_prep(inputs):
    """Fold conv+bn+biases; build the 8 per-core input maps."""
    f32 = np.float32
    q = np.ascontiguousarray(inputs["query"], dtype=f32)
    k = np.ascontiguousarray(inputs["key"], dtype=f32)
    v = np.ascontiguousarray(inputs["value"], dtype=f32)
    w3 = np.asarray(inputs["conv_w3"], f32)
    w5 = np.asarray(inputs["conv_w5"], f32)
    b3 = np.asarray(inputs["conv_b3"], f32)
    b5 = np.asarray(inputs["conv_b5"], f32)
    gam = np.asarray(inputs["bn_gamma"], f32)
    bet = np.asarray(inputs["bn_beta"], f32)
    mu = np.asarray(inputs["bn_mean"], f32)
    var = np.asarray(inputs["bn_var"], f32)
    wq = np.asarray(inputs["wq"], f32)
    bq = np.asarray(inputs["bq"], f32)
    wkl = np.asarray(inputs["wkl"], f32)
    bkl = np.asarray(inputs["bkl"], f32)
    wkg = np.asarray(inputs["wkg"], f32)
    bkg = np.asarray(inputs["bkg"], f32)
    wv = np.asarray(inputs["wv"], f32)
    bv = np.asarray(inputs["bv"], f32)
    wo = np.asarray(inputs["wo"], f32)
    bo = np.asarray(inputs["bo"], f32)

    assert not np.any(bq) and not np.any(bkg), "nonzero q/kg bias unsupported"

    s_bn = gam / np.sqrt(var + BN_EPS)
    shift = np.concatenate([b3, b5]) * s_bn + (bet - mu * s_bn)
    wkl_s = wkl * s_bn[None, :]
    A3 = np.einsum("oc,cit->oit", wkl_s[:, :D], w3)
    A5 = np.einsum("oc,cit->oit", wkl_s[:, D:], w5)
    W5c = A5.copy()
    W5c[:, :, 1:4] += A3
    bkl_eff = wkl @ shift + bkl
    wq_eff = wq / math.sqrt(DK)
    bo_eff = bo + wo @ (2.0 * bv)

    bf = BF16_NP
    ident = np.eye(128, dtype=bf)
    in_maps = []
    for c in range(N_CORES):
        b = c // 2
        hg = c % 2
        sel = slice(hg * DO, hg * DO + DO)
        wo_sel = wo.T[sel, :]
        wo2 = np.ascontiguousarray(
            wo_sel.reshape(2, 2, 64, D).transpose(1, 2, 0, 3).reshape(128, 2, D))
        in_maps.append({
            "xq": np.ascontiguousarray(q[b].T).astype(bf),
            "xk": np.ascontiguousarray(k[b].T).astype(bf),
            "xv": np.ascontiguousarray(v[b].T).astype(bf),
            "wq": np.ascontiguousarray(wq_eff.T[:, sel]).astype(bf),
            "wk5": np.ascontiguousarray(W5c.transpose(2, 1, 0)[:, :, sel]).astype(bf),
            "wkg": np.ascontiguousarray(wkg.T[:, sel]).astype(bf),
            "wv": np.ascontiguousarray(wv.T[:, sel]).astype(bf),
            "wo2": wo2.astype(bf),
            "bkl": np.ascontiguousarray(bkl_eff[sel]).astype(f32),
            "ident": ident,
        })
    return in_maps, bo_eff


def kernel(**inputs) -> np.ndarray:
    if "nc" not in _cache:
        _cache["nc"] = _build_program()
    nc = _cache["nc"]
    in_maps, bo_eff = _host_prep(inputs)
    res = bass_utils.run_bass_kernel_spmd(
        nc, in_maps, core_ids=list(range(N_CORES)))
    out = np.zeros((B, L, D), np.float32)
    for c in range(N_CORES):
        out[c // 2] += np.asarray(res.results[c]["out"], np.float32)
    out += bo_eff[None, None, :]
    return out
